# revision 1
# baseline (speedup 1.0000x reference)
"""Trainium2 Bass kernel for nn_CrossAttnBlockppTwoCams.

Sharding: 8 cores = 4 scene-groups x 2 pair-halves. Core (g, s) handles scene
group g (batch entries 4g..4g+3) and attention pairs {3s, 3s+1, 3s+2} of the 6
cross-camera pairs, all 4 heads each. With this split, each core produces two
COMPLETE output batch entries (4g+2s+0, 4g+2s+1) because the final 1x1-conv
channel blocks of those entries come exactly from this core's pairs.

Per core: GroupNorm (6 slot-inputs), q/k/v NIN projections (f32r matmuls),
6x4 = 12 attention units of [1024q x 1024k] with C=128 on partitions,
softmax via exp on ScalarE + ones-matmul partition reduce, final NIN into two
accumulators. Matmuls use float32r (TF32-like, 1 col/cycle); attention
probabilities are stored bf16; V^T is f32r via PE transpose.
"""
import sys
import os

sys.path.insert(0, '/opt/trn_rl_repo')

import numpy as np

B, C, HH, WW = 16, 128, 32, 32
HW = HH * WW
NH, COND, GROUPS, EPS = 4, 32, 32, 1e-6
SCALE = float(C) ** -0.5
PAIRS = [(0, 1), (1, 0), (2, 3), (3, 2), (0, 2), (2, 0)]  # (q cam, kv cam)

_PROG = None


def _build_nc(repeat=1):
    import concourse.bacc as bacc
    import concourse.tile as tile
    import concourse.mybir as mybir

    f32 = mybir.dt.float32
    f32r = mybir.dt.float32r
    bf16 = mybir.dt.bfloat16
    AF = mybir.ActivationFunctionType
    ALU = mybir.AluOpType
    X_AX = mybir.AxisListType.X

    nc = bacc.Bacc("TRN2", target_bir_lowering=False, debug=False, num_devices=8)

    d_xq = nc.dram_tensor("xq", [3, C, HW], f32, kind="ExternalInput")
    d_xkv = nc.dram_tensor("xkv", [3, C, HW], f32, kind="ExternalInput")
    d_qc = nc.dram_tensor("qc", [3, COND, HW], f32r, kind="ExternalInput")
    d_kc = nc.dram_tensor("kc", [3, COND, HW], f32r, kind="ExternalInput")
    d_gnv = nc.dram_tensor("gnv", [C, 2], f32, kind="ExternalInput")
    d_wA = nc.dram_tensor("wA", [C, 3 * 512], f32r, kind="ExternalInput")
    d_wB = nc.dram_tensor("wB", [COND, 3 * 512], f32r, kind="ExternalInput")
    d_bqkv = nc.dram_tensor("bqkv", [C, 12], f32, kind="ExternalInput")
    d_w3 = nc.dram_tensor("w3", [C, 12 * C], f32r, kind="ExternalInput")
    d_ident = nc.dram_tensor("ident", [C, C], f32, kind="ExternalInput")
    d_constr = nc.dram_tensor("constr", [C, C], f32r, kind="ExternalInput")
    d_gind = nc.dram_tensor("gind", [C, GROUPS], f32, kind="ExternalInput")
    d_gindT = nc.dram_tensor("gindT", [GROUPS, C], f32, kind="ExternalInput")
    d_out = nc.dram_tensor("out", [2, C, HW], f32, kind="ExternalOutput")

    with tile.TileContext(nc) as tc, nc.allow_low_precision(reason="f32r pipeline"):
        import contextlib
        ctx = contextlib.ExitStack()
        with ctx:
            cpool = ctx.enter_context(tc.tile_pool(name="consts", bufs=1))
            xpool = ctx.enter_context(tc.tile_pool(name="xp", bufs=2))
            hpool = ctx.enter_context(tc.tile_pool(name="hp", bufs=6))
            gns = ctx.enter_context(tc.tile_pool(name="gns", bufs=3))
            scr = ctx.enter_context(tc.tile_pool(name="scr", bufs=2))
            qpool = ctx.enter_context(tc.tile_pool(name="qp", bufs=6))
            kpool = ctx.enter_context(tc.tile_pool(name="kp", bufs=6))
            vpool = ctx.enter_context(tc.tile_pool(name="vp", bufs=2))
            vtpool = ctx.enter_context(tc.tile_pool(name="vtp", bufs=6))
            epool = ctx.enter_context(tc.tile_pool(name="ep", bufs=9))
            chpool = ctx.enter_context(tc.tile_pool(name="chp", bufs=4))
            opool = ctx.enter_context(tc.tile_pool(name="op", bufs=2))
            apool = ctx.enter_context(tc.tile_pool(name="ap", bufs=2))
            P1 = ctx.enter_context(tc.tile_pool(name="ps1", bufs=2, space="PSUM"))
            P2 = ctx.enter_context(tc.tile_pool(name="ps2", bufs=2, space="PSUM"))

            # ---- constants ----
            ident = cpool.tile([C, C], f32, tag="ident")
            nc.sync.dma_start(ident[:], d_ident[:])
            constr = cpool.tile([C, C], f32r, tag="constr")
            nc.sync.dma_start(constr[:], d_constr[:])
            ones_col = constr[:, 0:1]          # [128,1] ones (f32r)
            ones_row = constr[0:1, :]          # [1,128] ones (f32r)
            onesb = cpool.tile([C, 1], bf16, tag="onesb")
            nc.vector.tensor_copy(onesb[:], constr[:, 0:1].bitcast(f32))
            gind = cpool.tile([C, GROUPS], f32, tag="gind")
            nc.sync.dma_start(gind[:], d_gind[:])
            gindT = cpool.tile([GROUPS, C], f32, tag="gindT")
            nc.sync.dma_start(gindT[:], d_gindT[:])
            wA = cpool.tile([C, 3 * 512], f32r, tag="wA")
            nc.sync.dma_start(wA[:], d_wA[:])
            wB = cpool.tile([COND, 3 * 512], f32r, tag="wB")
            nc.sync.dma_start(wB[:], d_wB[:])
            bqkv = cpool.tile([C, 12], f32, tag="bqkv")
            nc.sync.dma_start(bqkv[:], d_bqkv[:])
            w3 = cpool.tile([C, 12 * C], f32r, tag="w3")
            nc.sync.dma_start(w3[:], d_w3[:])
            gnv = cpool.tile([C, 2], f32, tag="gnv")
            nc.sync.dma_start(gnv[:], d_gnv[:])
            epst = cpool.tile([GROUPS, 1], f32, tag="epst")
            nc.vector.memset(epst[:], EPS)
            qct = []
            kct = []
            for i in range(3):
                q_t = cpool.tile([COND, HW], f32r, tag=f"qc{i}")
                nc.sync.dma_start(q_t[:], d_qc[i])
                qct.append(q_t)
                k_t = cpool.tile([COND, HW], f32r, tag=f"kc{i}")
                nc.sync.dma_start(k_t[:], d_kc[i])
                kct.append(k_t)

            # ---- GroupNorm for the 6 slot-inputs ----
            def group_norm(src_ap):
                xt = xpool.tile([C, HW], f32, tag="xt")
                nc.sync.dma_start(xt[:], src_ap)
                stat = gns.tile([C, 2], f32, tag="stat")
                nc.vector.tensor_reduce(out=stat[:, 0:1], in_=xt[:], axis=X_AX, op=ALU.add)
                sq = scr.tile([C, HW], f32, tag="sq", bufs=1)
                nc.vector.tensor_tensor(out=sq[:], in0=xt[:], in1=xt[:], op=ALU.mult)
                nc.vector.tensor_reduce(out=stat[:, 1:2], in_=sq[:], axis=X_AX, op=ALU.add)
                ps_g = P2.tile([GROUPS, 2], f32, tag="tail")
                nc.tensor.matmul(ps_g[:], gind[:], stat[:], start=True, stop=True)
                mr = gns.tile([GROUPS, 2], f32, tag="mr")
                nc.vector.tensor_copy(mr[:, 0:1], ps_g[:, 0:1])
                mu2 = gns.tile([GROUPS, 1], f32, tag="mu2")
                nc.vector.tensor_tensor(out=mu2[:], in0=mr[:, 0:1], in1=mr[:, 0:1], op=ALU.mult)
                var = gns.tile([GROUPS, 1], f32, tag="var")
                nc.vector.tensor_tensor(out=var[:], in0=ps_g[:, 1:2], in1=mu2[:], op=ALU.subtract)
                lnv = gns.tile([GROUPS, 1], f32, tag="lnv")
                nc.scalar.activation(lnv[:], var[:], AF.Ln, bias=epst[:], scale=1.0)
                nc.scalar.activation(mr[:, 1:2], lnv[:], AF.Exp, scale=-0.5)
                ps_bc = P2.tile([C, 2], f32, tag="tail")
                nc.tensor.matmul(ps_bc[:], gindT[:], mr[:], start=True, stop=True)
                se = gns.tile([C, 1], f32, tag="se")
                nc.vector.tensor_tensor(out=se[:], in0=ps_bc[:, 1:2], in1=gnv[:, 0:1], op=ALU.mult)
                ms = gns.tile([C, 1], f32, tag="ms")
                nc.vector.tensor_tensor(out=ms[:], in0=ps_bc[:, 0:1], in1=se[:], op=ALU.mult)
                be = gns.tile([C, 1], f32, tag="be")
                nc.vector.tensor_tensor(out=be[:], in0=gnv[:, 1:2], in1=ms[:], op=ALU.subtract)
                ht = hpool.tile([C, HW], f32r, tag="ht")
                nc.vector.tensor_scalar(out=ht[:], in0=xt[:], scalar1=se[:], scalar2=be[:],
                                        op0=ALU.mult, op1=ALU.add)
                return ht

            for _rep in range(repeat):
              hq = [None] * 3
              hkv = [None] * 3
              for i in range(3):
                  hq[i] = group_norm(d_xq[i])
                  hkv[i] = group_norm(d_xkv[i])

              # ---- NIN projections + attention per slot ----
              def nin_head(proj, h, h_src, c_src, out_dt, pool, eng):
                  # proj: 0=q,1=k,2=v ; returns [128,1024] tile of dtype out_dt
                  wa = wA[:, proj * 512 + h * 128: proj * 512 + (h + 1) * 128]
                  wb = wB[:, proj * 512 + h * 128: proj * 512 + (h + 1) * 128]
                  ps = P1.tile([C, HW], f32, tag="mm")
                  for half in range(2):
                      fr = slice(half * 512, (half + 1) * 512)
                      nc.tensor.matmul(ps[:, fr], wa, h_src[:, fr], start=True, stop=False)
                      nc.tensor.matmul(ps[:, fr], wb, c_src[:, fr], start=False, stop=True)
                  t = pool.tile([C, HW], out_dt, tag="t")
                  bias = bqkv[:, proj * 4 + h: proj * 4 + h + 1]
                  if eng == "dve":
                      nc.vector.tensor_scalar_add(out=t[:], in0=ps[:], scalar1=bias)
                  else:
                      nc.scalar.activation(t[:], ps[:], AF.Identity, bias=bias, scale=1.0)
                  return t

              # ---- per-slot NIN + attention ----
              acc = [None, None]
              for i in range(3):
                  Qs = {}
                  Ks = {}
                  VTs = {}
                  for h in range(NH):
                      Qs[h] = nin_head(0, h, hq[i], qct[i], f32r, qpool, "dve" if h < 2 else "act")
                      Ks[h] = nin_head(1, h, hkv[i], kct[i], f32r, kpool, "dve" if h < 2 else "act")
                      vt_src = nin_head(2, h, hkv[i], kct[i], f32, vpool, "dve" if h < 2 else "act")
                      vt = vtpool.tile([C, HW], bf16, tag="vt")
                      for w in range(2):
                          ps_vt = P2.tile([C, 512], f32, tag="tail")
                          for blk in range(4):
                              kb = w * 4 + blk
                              nc.tensor.transpose(ps_vt[:, blk * 128:(blk + 1) * 128],
                                                  vt_src[:, kb * 128:(kb + 1) * 128], ident[:])
                          nc.vector.tensor_copy(vt[:, w * 512:(w + 1) * 512], ps_vt[:])
                      VTs[h] = vt

                  for h in range(NH):
                      u = 4 * i + h
                      q_t, k_t, vt_t = Qs[h], Ks[h], VTs[h]
                      E = []
                      for kt in range(8):
                          ps_s = P1.tile([C, HW], f32, tag="mm")
                          lhs = k_t[:, kt * 128:(kt + 1) * 128]
                          nc.tensor.matmul(ps_s[:, 0:512], lhs, q_t[:, 0:512], start=True, stop=True)
                          nc.tensor.matmul(ps_s[:, 512:1024], lhs, q_t[:, 512:1024], start=True, stop=True)
                          e_t = epool.tile([C, HW], bf16, tag="et")
                          nc.scalar.activation(e_t[:], ps_s[:], AF.Exp, scale=SCALE)
                          E.append(e_t)
                      ps_o = P2.tile([C, HW], f32, tag="tail")
                      for kt in range(8):
                          st, sp = kt == 0, kt == 7
                          lhs = vt_t[:, kt * 128:(kt + 1) * 128]
                          nc.tensor.matmul(ps_o[:, 0:512], lhs, E[kt][:, 0:512], start=st, stop=sp)
                          nc.tensor.matmul(ps_o[:, 512:1024], lhs, E[kt][:, 512:1024], start=st, stop=sp)
                      # denominator: 2-level bf16 pairwise tree + PE ones-reduce
                      quads = []
                      for t in range(2):
                          p0 = chpool.tile([C, HW], bf16, tag="chain")
                          nc.vector.tensor_tensor(out=p0[:], in0=E[4 * t][:], in1=E[4 * t + 1][:], op=ALU.add)
                          p1 = chpool.tile([C, HW], bf16, tag="chain")
                          nc.vector.tensor_tensor(out=p1[:], in0=E[4 * t + 2][:], in1=E[4 * t + 3][:], op=ALU.add)
                          nc.vector.tensor_tensor(out=p0[:], in0=p0[:], in1=p1[:], op=ALU.add)
                          quads.append(p0)
                      ps_d = P2.tile([1, HW], f32, tag="tail")
                      for half in range(2):
                          fr = slice(half * 512, (half + 1) * 512)
                          nc.tensor.matmul(ps_d[0:1, fr], onesb[:], quads[0][:, fr], start=True, stop=False)
                          nc.tensor.matmul(ps_d[0:1, fr], onesb[:], quads[1][:, fr], start=False, stop=True)
                      r_row = chpool.tile([1, HW], f32r, tag="chain")
                      nc.vector.reciprocal(out=r_row[:], in_=ps_d[0:1, :])
                      ps_b = P2.tile([C, HW], f32, tag="tail")
                      nc.tensor.matmul(ps_b[:, 0:512], ones_row, r_row[0:1, 0:512], start=True, stop=True)
                      nc.tensor.matmul(ps_b[:, 512:1024], ones_row, r_row[0:1, 512:1024], start=True, stop=True)
                      bsb = scr.tile([C, HW], f32, tag="bsb")
                      nc.scalar.copy(bsb[:], ps_b[:])
                      o_sb = opool.tile([C, HW], f32r, tag="osb")
                      nc.vector.tensor_tensor(out=o_sb[:], in0=ps_o[:], in1=bsb[:], op=ALU.mult)
                      ps_n = P2.tile([C, HW], f32, tag="tail")
                      w3u = w3[:, u * 128:(u + 1) * 128]
                      nc.tensor.matmul(ps_n[:, 0:512], w3u, o_sb[:, 0:512], start=True, stop=True)
                      nc.tensor.matmul(ps_n[:, 512:1024], w3u, o_sb[:, 512:1024], start=True, stop=True)
                      j = 0 if u < 6 else 1
                      if u % 6 == 0:
                          acc_t = apool.tile([C, HW], f32, tag="acc")
                          acc[j] = acc_t
                          nc.vector.tensor_copy(acc_t[:], ps_n[:])
                      else:
                          nc.vector.tensor_tensor(out=acc[j][:], in0=acc[j][:], in1=ps_n[:], op=ALU.add)
                      if u % 6 == 5:
                          nc.sync.dma_start(d_out[j], acc[j][:])

    nc.compile()
    return nc


def _get_prog(repeat=1):
    global _PROG
    if repeat != 1:
        return _build_nc(repeat=repeat)
    if _PROG is None:
        _PROG = _build_nc()
    return _PROG


def _make_in_maps(x, q_cond, k_a_cond, k_b_cond, gn_scale, gn_bias,
                  W0, b0, W1, b1, W2, b2, W3, b3):
    f4 = np.float32
    x = np.ascontiguousarray(x, f4).reshape(B, C, HW)
    q_cond = np.ascontiguousarray(q_cond, f4).reshape(B // 2, COND, HW)
    k_a = np.ascontiguousarray(k_a_cond, f4).reshape(B // 2, COND, HW)
    k_b = np.ascontiguousarray(k_b_cond, f4).reshape(B // 2, COND, HW)

    wA = np.concatenate([W0[:C], W1[:C], W2[:C]], axis=1).astype(f4)        # [128, 1536]
    wB = np.concatenate([W0[C:], W1[C:], W2[C:]], axis=1).astype(f4)        # [32, 1536]
    bqkv = np.stack([b0.reshape(NH, C), b1.reshape(NH, C), b2.reshape(NH, C)]) \
             .reshape(12, C).T.astype(f4).copy()                            # [128, 12]
    gnv = np.stack([gn_scale, gn_bias], axis=1).astype(f4)                  # [128, 2]
    ident = np.eye(C, dtype=f4)
    constr = np.zeros((C, C), f4)
    constr[:, 0] = 1.0
    constr[0, :] = 1.0
    gind = np.zeros((C, GROUPS), f4)
    for c in range(C):
        gind[c, c // (C // GROUPS)] = 1.0 / (C // GROUPS * HW)
    gindT = np.zeros((GROUPS, C), f4)
    for c in range(C):
        gindT[c // (C // GROUPS), c] = 1.0

    def qcs(b):
        return q_cond[b // 2]

    def kcs(b):
        return (k_a if b % 2 == 0 else k_b)[b // 2]

    in_maps = []
    for core in range(8):
        g, s = core // 2, core % 2
        plist = [3 * s + 0, 3 * s + 1, 3 * s + 2]
        xq = np.stack([x[4 * g + PAIRS[p][0]] for p in plist])
        xkv = np.stack([x[4 * g + PAIRS[p][1]] for p in plist])
        qc = np.stack([qcs(4 * g + PAIRS[p][0]) for p in plist])
        kc = np.stack([kcs(4 * g + PAIRS[p][1]) for p in plist])
        w3l = np.zeros((C, 12 * C), f4)
        for u in range(12):
            i, h = u // 4, u % 4
            f = 512 * plist[i] + 128 * h
            r = f % 768
            w3l[:, u * C:(u + 1) * C] = W3[r:r + C, :]
        in_maps.append({
            "xq": xq, "xkv": xkv, "qc": qc, "kc": kc, "gnv": gnv,
            "wA": wA, "wB": wB, "bqkv": bqkv, "w3": w3l,
            "ident": ident, "constr": constr, "gind": gind, "gindT": gindT,
        })
    return in_maps


def _assemble(results, x, b3):
    x = np.ascontiguousarray(x, np.float32)
    out = np.empty_like(x)
    for core in range(8):
        g, s = core // 2, core % 2
        o = results[core]["out"].reshape(2, C, HH, WW)
        for j in range(2):
            b = 4 * g + 2 * s + j
            out[b] = x[b] + o[j] + b3[:, None, None].astype(np.float32)
    return out


def kernel(**inputs):
    from concourse.bass_utils import run_bass_kernel_spmd
    nc = _get_prog()
    ins = {k: np.asarray(v) for k, v in inputs.items()}
    in_maps = _make_in_maps(**ins)
    res = run_bass_kernel_spmd(nc, in_maps, core_ids=list(range(8)))
    return _assemble(res.results, ins["x"], ins["b3"])



# revision 24
# speedup vs baseline: 1.5219x; 1.5219x over previous
"""Trainium2 Bass kernel for nn_CrossAttnBlockppTwoCams.

Sharding: 8 cores = 4 scene-groups x 2 head-halves. Core (g, s) handles scene
group g (batch entries 4g..4g+3) and heads {2s, 2s+1} of all 6 cross-camera
attention pairs -> 12 attention units per core. Each core emits 6 per-pair
partial accumulators (post-W3); the host sums them into the 4 output entries
of the group (each entry receives 1.5 pairs' worth of channels).

Device pipeline per core:
  - GroupNorm for the 4 cameras (bf16 x, stats via DVE reduces + tiny PE
    matmuls, Ln/Exp batched over cams so only 2 act-table loads happen).
  - NIN projections use only the 128-channel h contraction on the PE; the
    32-channel cond contraction plus biases are precomputed on the host and
    added during the PSUM->SBUF drain (DVE/Pool), so no 32-partition matmuls.
  - V is projected directly in transposed (key-partitioned) layout by
    swapping matmul roles, eliminating PE transposes.
  - Attention: f32r q/k scores (8x[128,1024] PSUM tiles), exp on Act (bf16,
    the only large Act work: 96 x 1024-col tiles), softmax denominator via a
    bf16 pairwise tree on DVE + GPSIMD partition_all_reduce (fused
    reduce+broadcast, no PE/PSUM involvement), normalization via DVE divide,
    bf16 AV, W3 accumulated in PSUM across the 2 heads of a pair.
  - Emission is software-pipelined: scores of unit u+1 are emitted before
    AV/W3 of unit u so the PE never waits on the (rate-limiting) Act engine.
"""
import sys

sys.path.insert(0, '/opt/trn_rl_repo')

import numpy as np
import ml_dtypes

B, C, HH, WW = 16, 128, 32, 32
HW = HH * WW
NH, COND, GROUPS, EPS = 4, 32, 32, 1e-6
SCALE = float(C) ** -0.5
PAIRS = [(0, 1), (1, 0), (2, 3), (3, 2), (0, 2), (2, 0)]  # (q cam, kv cam)
BF = ml_dtypes.bfloat16

_PROG = None


def _build_nc():
    import concourse.bacc as bacc
    import concourse.tile as tile
    import concourse.mybir as mybir
    import concourse.bass_isa as bass_isa

    f32 = mybir.dt.float32
    f32r = mybir.dt.float32r
    bf16 = mybir.dt.bfloat16
    AF = mybir.ActivationFunctionType
    ALU = mybir.AluOpType
    X_AX = mybir.AxisListType.X

    nc = bacc.Bacc("TRN2", target_bir_lowering=False, debug=False, num_devices=8)

    d_x = nc.dram_tensor("x4", [4, C, HW], bf16, kind="ExternalInput")
    d_cqk = nc.dram_tensor("cqk", [16, C, HW], bf16, kind="ExternalInput")
    d_kcp = nc.dram_tensor("kcp", [4, COND + 1, HW], bf16, kind="ExternalInput")
    d_w2c = nc.dram_tensor("w2c", [COND + 1, 256], bf16, kind="ExternalInput")
    d_wqk = nc.dram_tensor("wqk", [C, 512], bf16, kind="ExternalInput")
    d_w2 = nc.dram_tensor("w2", [C, 256], bf16, kind="ExternalInput")
    d_w3 = nc.dram_tensor("w3", [C, 12 * C], bf16, kind="ExternalInput")
    d_gnv = nc.dram_tensor("gnv", [C, 2], f32, kind="ExternalInput")
    d_gind = nc.dram_tensor("gind", [C, GROUPS], bf16, kind="ExternalInput")
    d_gindT = nc.dram_tensor("gindT", [GROUPS, C], bf16, kind="ExternalInput")
    d_out = nc.dram_tensor("out", [6, C, HW], f32, kind="ExternalOutput")

    with tile.TileContext(nc) as tc, nc.allow_low_precision(reason="bf16 pipeline"):
        import contextlib
        ctx = contextlib.ExitStack()
        with ctx:
            cpool = ctx.enter_context(tc.tile_pool(name="consts", bufs=1))
            xpool = ctx.enter_context(tc.tile_pool(name="xp", bufs=1))
            sqpool = ctx.enter_context(tc.tile_pool(name="sqp", bufs=2))
            stpool = ctx.enter_context(tc.tile_pool(name="stp", bufs=2))
            smpool = ctx.enter_context(tc.tile_pool(name="smp", bufs=2))
            sbpool = ctx.enter_context(tc.tile_pool(name="sbp", bufs=2))
            hpool = ctx.enter_context(tc.tile_pool(name="hp", bufs=1))
            condp = ctx.enter_context(tc.tile_pool(name="condp", bufs=8))
            qkpool = ctx.enter_context(tc.tile_pool(name="qkp", bufs=1))
            vtpool = ctx.enter_context(tc.tile_pool(name="vtp", bufs=1))
            epool = ctx.enter_context(tc.tile_pool(name="ep", bufs=16))
            fpool = ctx.enter_context(tc.tile_pool(name="fp", bufs=6))
            dpool = ctx.enter_context(tc.tile_pool(name="dp", bufs=2))
            opool = ctx.enter_context(tc.tile_pool(name="op", bufs=6))
            apool = ctx.enter_context(tc.tile_pool(name="ap", bufs=2))
            P1 = ctx.enter_context(tc.tile_pool(name="ps1", bufs=2, space="PSUM"))
            PB = ctx.enter_context(tc.tile_pool(name="psb", bufs=1, space="PSUM"))
            PA = ctx.enter_context(tc.tile_pool(name="psa", bufs=1, space="PSUM"))

            # ---- constants (x + GN path first; w3 last) ----
            xt = [None] * 4
            for m in range(4):
                xt[m] = xpool.tile([C, HW], bf16, tag=f"xt{m}", name=f"xt{m}")
                nc.sync.dma_start(xt[m][:], d_x[m])
            gnv = cpool.tile([C, 2], f32, tag="gnv")
            nc.sync.dma_start(gnv[:], d_gnv[:])
            gind = cpool.tile([C, GROUPS], bf16, tag="gind")
            nc.sync.dma_start(gind[:], d_gind[:])
            gindT = cpool.tile([GROUPS, C], bf16, tag="gindT")
            nc.sync.dma_start(gindT[:], d_gindT[:])
            wqk = cpool.tile([C, 512], bf16, tag="wqk")
            nc.sync.dma_start(wqk[:], d_wqk[:])
            w2 = cpool.tile([C, 256], bf16, tag="w2")
            nc.sync.dma_start(w2[:], d_w2[:])
            w2c = cpool.tile([COND + 1, 256], bf16, tag="w2c")
            nc.sync.dma_start(w2c[:], d_w2c[:])
            kcp = [None] * 4
            for m in range(4):
                kcp[m] = cpool.tile([COND + 1, HW], bf16, tag=f"kcp{m}", name=f"kcp{m}")
                nc.sync.dma_start(kcp[m][:], d_kcp[m])
            epst = cpool.tile([GROUPS, 1], f32, tag="epst")
            nc.vector.memset(epst[:], EPS)
            # cond tiles: issue all DMAs now, ordered by first use (cam order);
            # the pool depth paces the stream against drain consumption.
            ct_tiles = {}
            CT_ORDER = [(0, 0, 0), (1, 1, 0), (0, 0, 1), (1, 1, 1),
                        (1, 0, 0), (0, 1, 0), (1, 0, 1), (0, 1, 1),
                        (2, 0, 0), (3, 1, 0), (2, 0, 1), (3, 1, 1),
                        (3, 0, 0), (2, 1, 0), (3, 0, 1), (2, 1, 1)]
            for (m, proj, i) in CT_ORDER:
                ct = condp.tile([C, HW], bf16, tag="ct", name=f"ct{m}{proj}{i}")
                nc.sync.dma_start(ct[:], d_cqk[proj * 8 + m * 2 + i])
                ct_tiles[(m, proj, i)] = ct
            w3 = cpool.tile([C, 12 * C], bf16, tag="w3")
            nc.sync.dma_start(w3[:], d_w3[:])

            # GN small PSUM lives in one acc-shaped tile (regions), freed
            # before the attention-phase acc allocations cycle the same buf.
            gn_ps = PA.tile([C, HW], f32, tag="acc", name="gn_ps")

            # GN stats: plain sum on DVE, sum-of-squares on the (idle) Act
            # engine via Square+accum_out, tiny group matmul.
            for m in range(4):
                st = stpool.tile([C, 2], bf16, tag="st", name=f"st{m}", bufs=4)
                scr = sqpool.tile([C, HW], bf16, tag="sq")
                nc.scalar.activation(scr[:], xt[m][:], AF.Identity, accum_out=st[:, 0:1])
                scr2 = sqpool.tile([C, HW], bf16, tag="sq")
                nc.scalar.activation(scr2[:], xt[m][:], AF.Square, accum_out=st[:, 1:2])
                nc.tensor.matmul(gn_ps[0:GROUPS, 2 * m:2 * m + 2], gind[:], st[:],
                                 start=True, stop=True)

            # GN finish per cam: rstd = sqrt(1/(var+eps)) (no Ln/Exp -> only
            # sqrt+exp act tables are ever loaded, 2 table loads total).
            h_t = [None] * 4
            for m in range(4):
                mu = smpool.tile([GROUPS, 1], f32, tag="mu")
                nc.vector.tensor_copy(mu[:], gn_ps[0:GROUPS, 2 * m:2 * m + 1])
                mu2 = smpool.tile([GROUPS, 1], f32, tag="mu2")
                nc.vector.tensor_tensor(out=mu2[:], in0=mu[:], in1=mu[:], op=ALU.mult)
                vpe = smpool.tile([GROUPS, 1], f32, tag="vpe")
                nc.vector.tensor_tensor(out=vpe[:], in0=gn_ps[0:GROUPS, 2 * m + 1:2 * m + 2],
                                        in1=mu2[:], op=ALU.subtract)
                rv = smpool.tile([GROUPS, 1], f32, tag="rv")
                nc.vector.tensor_scalar_add(out=rv[:], in0=vpe[:], scalar1=epst[:])
                nc.vector.reciprocal(out=rv[:], in_=rv[:])
                rstd = smpool.tile([GROUPS, 1], f32, tag="rstd")
                nc.scalar.activation(rstd[:], rv[:], AF.Sqrt)
                bc_in = smpool.tile([GROUPS, 2], bf16, tag="bcin")
                nc.vector.tensor_copy(bc_in[:, 0:1], rstd[:])
                nc.vector.tensor_copy(bc_in[:, 1:2], mu[:])
                nc.tensor.matmul(gn_ps[:, 16 + 2 * m:18 + 2 * m], gindT[:], bc_in[:],
                                 start=True, stop=True)
                se = sbpool.tile([C, 1], f32, tag="se")
                nc.vector.tensor_tensor(out=se[:], in0=gn_ps[:, 16 + 2 * m:17 + 2 * m],
                                        in1=gnv[:, 0:1], op=ALU.mult)
                ms = smpool.tile([C, 1], f32, tag="ms")
                nc.vector.tensor_tensor(out=ms[:], in0=gn_ps[:, 17 + 2 * m:18 + 2 * m],
                                        in1=se[:], op=ALU.mult)
                be = sbpool.tile([C, 1], f32, tag="be")
                nc.vector.tensor_tensor(out=be[:], in0=gnv[:, 1:2], in1=ms[:], op=ALU.subtract)
                ht = hpool.tile([C, HW], bf16, tag=f"ht{m}", name=f"ht{m}")
                nc.vector.tensor_scalar(out=ht[:], in0=xt[m][:], scalar1=se[:], scalar2=be[:],
                                        op0=ALU.mult, op1=ALU.add)
                h_t[m] = ht

            q_sb = [[None] * 2 for _ in range(4)]
            k_sb = [[None] * 2 for _ in range(4)]
            vt_sb = [[None] * 2 for _ in range(4)]
            drain_rr = [0]

            def emit_nin_qk(m, proj, i, pool, tagn, vt_eng):
                ht = h_t[m]
                ps = pool.tile([C, HW], f32, tag=tagn, name="ps_nin")
                wblk = wqk[:, (proj * 2 + i) * 128:(proj * 2 + i + 1) * 128]
                for hf in range(2):
                    fr = slice(hf * 512, (hf + 1) * 512)
                    nc.tensor.matmul(ps[:, fr], wblk, ht[:, fr], start=True, stop=True)
                ct = ct_tiles[(m, proj, i)]
                dt_t = bf16
                t = qkpool.tile([C, HW], dt_t, tag=f"qk{m}_{proj}_{i}",
                                name=f"qk{m}_{proj}_{i}")
                nc.vector.tensor_tensor(out=t[:], in0=ps[:], in1=ct[:], op=ALU.add)
                (q_sb if proj == 0 else k_sb)[m][i] = t

            def emit_nin_vt(m, i, pool, tagn, vt_eng):
                # vT NIN: transposed roles -> output lands key-partitioned.
                # Cond+bias contraction also on the PE (33-partition padded),
                # so the drain is a plain copy.
                ht = h_t[m]
                ps = pool.tile([128, HW], f32, tag=tagn, name="ps_vt")
                for blk in range(8):
                    fr = slice(blk * 128, (blk + 1) * 128)
                    nc.tensor.matmul(ps[:, fr], ht[:, fr], w2[:, i * 128:(i + 1) * 128],
                                     start=True, stop=False)
                    nc.tensor.matmul(ps[:, fr], kcp[m][:, fr], w2c[:, i * 128:(i + 1) * 128],
                                     start=False, stop=True)
                vt = vtpool.tile([128, HW], bf16, tag=f"vt{m}_{i}", name=f"vt{m}_{i}")
                if vt_eng == 'act':
                    nc.scalar.activation(vt[:], ps[:], AF.Identity)
                else:
                    nc.vector.tensor_copy(vt[:], ps[:])
                vt_sb[m][i] = vt

            # cams 0/1 up front (prologue), rotating 3 PSUM bufs; vt drains on
            # the idle Act engine.
            rot = [(P1, "mm"), (P1, "mm"), (PB, "po")]
            nn = 0
            for (mq, mk) in ((0, 1), (1, 0)):
                for i in range(2):
                    for kind, mm_, pj in (('qk', mq, 0), ('qk', mk, 1), ('vt', mk, None)):
                        pool, tagn = rot[nn % 3]
                        nn += 1
                        if kind == 'qk':
                            emit_nin_qk(mm_, pj, i, pool, tagn, 'act')
                        else:
                            emit_nin_vt(mm_, i, pool, tagn, 'act')

            # cams 2/3: deferred into attention slots 0-3 (PA + po PSUM slots,
            # which are free until the first W3 closes at slot 4). Ordered by
            # first use: pair 2 needs (c2 q, c3 k/vt); pair 3 the reverse.
            deferred = []
            for (mq, mk) in ((2, 3), (3, 2)):
                for i in range(2):
                    deferred.append(lambda pool, tagn, mq=mq, i=i:
                                    emit_nin_qk(mq, 0, i, pool, tagn, None))
                    deferred.append(lambda pool, tagn, mk=mk, i=i:
                                    emit_nin_qk(mk, 1, i, pool, tagn, None))
                    deferred.append(lambda pool, tagn, mk=mk, i=i:
                                    emit_nin_vt(mk, i, pool, tagn, 'pool'))

            # ---- attention: 6 pairs x 2 heads, software-pipelined ----
            # Per slot u: scores/exp/folds of unit u interleaved (on the PE
            # stream) with AV chunks of unit u-1, so the PE fills the gaps
            # while Act paces the pipeline. W3+accumulate+drain close per
            # PAIR on a schedule that keeps the single acc PSUM buf free
            # during slots 0-3 (used by the deferred NIN).
            osb = {}

            def close_pair(pr, last_osb=None):
                accp = PA.tile([C, HW], f32, tag="acc", name=f"acc{pr}")
                for uu in (2 * pr, 2 * pr + 1):
                    ob = osb[uu] if last_osb is None or uu != 2 * pr + 1 else last_osb
                    w3u = w3[:, uu * 128:(uu + 1) * 128]
                    st, sp = uu == 2 * pr, uu == 2 * pr + 1
                    nc.tensor.matmul(accp[:, 0:512], w3u, ob[:, 0:512], start=st, stop=sp,
                                     skip_group_check=True)
                    nc.tensor.matmul(accp[:, 512:1024], w3u, ob[:, 512:1024], start=st, stop=sp,
                                     skip_group_check=True)
                asb = apool.tile([C, HW], f32, tag="asb", name=f"asb{pr}")
                nc.vector.tensor_copy(asb[:], accp[:])
                nc.sync.dma_start(d_out[pr], asb[:])

            def emit_slot(u, prev, hooks, w3_pairs, self_tail=False):
                p, i = u // 2, u % 2
                qc, kc = PAIRS[p]
                qs, ks = q_sb[qc][i], k_sb[kc][i]
                if prev is not None:
                    pu, pE, pdbc = prev
                    pvts = vt_sb[PAIRS[pu // 2][1]][pu % 2]
                    ps_o = PB.tile([C, HW], f32, tag="po", name="ps_o")

                def av_chunk(k0, k1):
                    if prev is None:
                        return
                    for kt in range(k0, k1):
                        st, sp = kt == 0, kt == 7
                        lhs = pvts[:, kt * 128:(kt + 1) * 128]
                        nc.tensor.matmul(ps_o[:, 0:512], lhs, pE[kt][:, 0:512], start=st, stop=sp)
                        nc.tensor.matmul(ps_o[:, 512:1024], lhs, pE[kt][:, 512:1024],
                                         start=st, stop=sp)

                E = []
                fs = {}

                def sc(kt):
                    ps_s = P1.tile([C, HW], f32, tag="mm", name="ps_s")
                    lhs = ks[:, kt * 128:(kt + 1) * 128]
                    nc.tensor.matmul(ps_s[:, 0:512], lhs, qs[:, 0:512], start=True, stop=True)
                    nc.tensor.matmul(ps_s[:, 512:1024], lhs, qs[:, 512:1024], start=True, stop=True)
                    e_t = epool.tile([C, HW], bf16, tag="et")
                    nc.scalar.activation(e_t[:], ps_s[:], AF.Exp, scale=SCALE)
                    E.append(e_t)

                def fold(a, b, dst=None, eng=None):
                    e = eng or nc.vector
                    if dst is None:
                        dst = fpool.tile([C, HW], bf16, tag="f", name="fold")
                        e.tensor_tensor(out=dst[:], in0=a[:], in1=b[:], op=ALU.add)
                    else:
                        e.tensor_tensor(out=dst[:], in0=dst[:], in1=a[:], op=ALU.add)
                    return dst

                sc(0)
                sc(1)
                fs['f01'] = fold(E[0], E[1])
                if len(hooks) > 0:
                    hooks[0](PA, "acc")
                av_chunk(0, 3)
                sc(2)
                sc(3)
                fs['f23'] = fold(E[2], E[3], eng=nc.gpsimd)
                fold(fs['f23'], None, dst=fs['f01'])
                av_chunk(3, 6)
                sc(4)
                sc(5)
                fs['f45'] = fold(E[4], E[5], eng=nc.gpsimd)
                av_chunk(6, 8)
                o_sb = None
                if prev is not None:
                    o_sb = opool.tile([C, HW], bf16, tag="osb", name="o_sb")
                    nc.vector.tensor_tensor(out=o_sb[:], in0=ps_o[:], in1=pdbc[:], op=ALU.mult)
                    osb[pu] = o_sb
                if len(hooks) > 1:
                    hooks[1](PB, "po")
                if self_tail:
                    ps_os = PB.tile([C, HW], f32, tag="po", name="ps_os")

                    def av_self(k0, k1):
                        for kt in range(k0, k1):
                            st, sp = kt == 0, kt == 7
                            lhs = vt_sb[kc][i][:, kt * 128:(kt + 1) * 128]
                            nc.tensor.matmul(ps_os[:, 0:512], lhs, E[kt][:, 0:512],
                                             start=st, stop=sp)
                            nc.tensor.matmul(ps_os[:, 512:1024], lhs, E[kt][:, 512:1024],
                                             start=st, stop=sp)
                    av_self(0, 4)
                sc(6)
                if self_tail:
                    av_self(4, 6)
                sc(7)
                fs['f67'] = fold(E[6], E[7])
                fold(fs['f67'], None, dst=fs['f45'])
                fold(fs['f45'], None, dst=fs['f01'])
                dbc = dpool.tile([C, HW], f32, tag="dbc", name="dbc")
                nc.gpsimd.partition_all_reduce(dbc[:], fs['f01'][:], channels=128,
                                               reduce_op=bass_isa.ReduceOp.add)
                nc.vector.reciprocal(out=dbc[:], in_=dbc[:])
                if len(hooks) > 2:
                    hooks[2](PA, "acc")
                for pr in w3_pairs:
                    close_pair(pr)
                if self_tail:
                    av_self(6, 8)
                    o_sbs = opool.tile([C, HW], bf16, tag="osb", name="o_sbs")
                    nc.vector.tensor_tensor(out=o_sbs[:], in0=ps_os[:], in1=dbc[:], op=ALU.mult)
                    close_pair(5, last_osb=o_sbs)
                return E, dbc

            W3_SCHED = {4: [0], 6: [1, 2], 8: [3], 10: [4]}
            pend = None
            for u in range(12):
                hooks = deferred[3 * u:3 * u + 3] if u < 4 else []
                E, dbc = emit_slot(u, pend, hooks, W3_SCHED.get(u, []),
                                   self_tail=(u == 11))
                pend = (u, E, dbc)

    nc.compile()
    return nc


def _get_prog():
    global _PROG
    if _PROG is None:
        _PROG = _build_nc()
    return _PROG


def _pack_host(x, q_cond, k_a_cond, k_b_cond, gn_scale, gn_bias,
               W0, b0, W1, b1, W2, b2, W3, b3):
    f4 = np.float32
    x = np.ascontiguousarray(x, f4).reshape(B, C, HW)
    q_cs = np.repeat(np.ascontiguousarray(q_cond, f4).reshape(B // 2, COND, HW), 2, axis=0)
    k_cs = np.stack([np.ascontiguousarray(k_a_cond, f4).reshape(B // 2, COND, HW),
                     np.ascontiguousarray(k_b_cond, f4).reshape(B // 2, COND, HW)],
                    axis=1).reshape(B, COND, HW)

    # cond contributions (+bias), f32 compute then bf16
    # q/k: [B, head, C, HW] for the 4 heads; vT: [B, head, HW, C] -> repacked
    condq = np.einsum('bcq,ck->bkq', q_cs, W0[C:].astype(f4)) + b0[None, :, None]
    condk = np.einsum('bcq,ck->bkq', k_cs, W1[C:].astype(f4)) + b1[None, :, None]
    condq = condq.reshape(B, NH, C, HW)
    condk = condk.reshape(B, NH, C, HW)

    gind = np.zeros((C, GROUPS), f4)
    for c in range(C):
        gind[c, c // (C // GROUPS)] = 1.0 / (C // GROUPS * HW)
    gindT = np.zeros((GROUPS, C), f4)
    for c in range(C):
        gindT[c // (C // GROUPS), c] = 1.0
    gnv = np.stack([np.asarray(gn_scale, f4), np.asarray(gn_bias, f4)], axis=1)

    in_maps = []
    for core in range(8):
        g, s = core // 2, core % 2
        hsel = [2 * s, 2 * s + 1]
        cams = [4 * g + m for m in range(4)]
        x4 = x[cams].astype(BF)
        cqk = np.empty((16, C, HW), BF)
        for m in range(4):
            for i in range(2):
                cqk[m * 2 + i] = condq[cams[m], hsel[i]].astype(BF)
                cqk[8 + m * 2 + i] = condk[cams[m], hsel[i]].astype(BF)
        kcp = np.ones((4, COND + 1, HW), f4)
        kcp[:, :COND] = k_cs[cams]
        w2c = np.zeros((COND + 1, 256), f4)
        for i in range(2):
            w2c[:COND, i * 128:(i + 1) * 128] = W2[C:, 128 * hsel[i]:128 * hsel[i] + 128]
            w2c[COND, i * 128:(i + 1) * 128] = b2[128 * hsel[i]:128 * hsel[i] + 128]
        wqk = np.concatenate([W0[:C, 128 * hsel[0]:128 * hsel[0] + 128],
                              W0[:C, 128 * hsel[1]:128 * hsel[1] + 128],
                              W1[:C, 128 * hsel[0]:128 * hsel[0] + 128],
                              W1[:C, 128 * hsel[1]:128 * hsel[1] + 128]], axis=1).astype(BF)
        w2m = np.concatenate([W2[:C, 128 * hsel[0]:128 * hsel[0] + 128],
                              W2[:C, 128 * hsel[1]:128 * hsel[1] + 128]], axis=1).astype(BF)
        w3l = np.zeros((C, 12 * C), f4)
        for p in range(6):
            for i in range(2):
                u = p * 2 + i
                ch = 512 * p + 128 * hsel[i]
                r = ch % 768
                w3l[:, u * C:(u + 1) * C] = W3[r:r + C, :]
        in_maps.append({
            "x4": x4, "cqk": cqk, "kcp": kcp.astype(BF), "w2c": w2c.astype(BF),
            "wqk": wqk, "w2": w2m, "w3": w3l.astype(BF),
            "gnv": gnv, "gind": gind.astype(BF), "gindT": gindT.astype(BF),
        })
    return in_maps


def _assemble(results, x, b3):
    x = np.ascontiguousarray(x, np.float32)
    out = x + np.asarray(b3, np.float32)[None, :, None, None]
    for core in range(8):
        g, s = core // 2, core % 2
        o = results[core]["out"].reshape(6, C, HH, WW)
        for p in range(6):
            j = (512 * p + 256 * s) // 768
            out[4 * g + j] += o[p]
    return out


def kernel(**inputs):
    from concourse.bass_utils import run_bass_kernel_spmd
    nc = _get_prog()
    ins = {k: np.asarray(v) for k, v in inputs.items()}
    in_maps = _pack_host(**ins)
    res = run_bass_kernel_spmd(nc, in_maps, core_ids=list(range(8)))
    return _assemble(res.results, ins["x"], ins["b3"])


# revision 31
# speedup vs baseline: 1.5493x; 1.0180x over previous
"""Trainium2 Bass kernel for nn_CrossAttnBlockppTwoCams.

Sharding: 8 cores = 4 scene-groups x 2 head-halves. Core (g, s) handles scene
group g (batch entries 4g..4g+3) and heads {2s, 2s+1} of all 6 cross-camera
attention pairs -> 12 attention units per core. Each core emits 6 per-pair
partial accumulators (post-W3); the host sums them into the 4 output entries
of the group (each entry receives 1.5 pairs' worth of channels).

Device pipeline per core:
  - GroupNorm for the 4 cameras (bf16 x, stats via DVE reduces + tiny PE
    matmuls, Ln/Exp batched over cams so only 2 act-table loads happen).
  - NIN projections use only the 128-channel h contraction on the PE; the
    32-channel cond contraction plus biases are precomputed on the host and
    added during the PSUM->SBUF drain (DVE/Pool), so no 32-partition matmuls.
  - V is projected directly in transposed (key-partitioned) layout by
    swapping matmul roles, eliminating PE transposes.
  - Attention: f32r q/k scores (8x[128,1024] PSUM tiles), exp on Act (bf16,
    the only large Act work: 96 x 1024-col tiles), softmax denominator via a
    bf16 pairwise tree on DVE + GPSIMD partition_all_reduce (fused
    reduce+broadcast, no PE/PSUM involvement), normalization via DVE divide,
    bf16 AV, W3 accumulated in PSUM across the 2 heads of a pair.
  - Emission is software-pipelined: scores of unit u+1 are emitted before
    AV/W3 of unit u so the PE never waits on the (rate-limiting) Act engine.
"""
import sys

sys.path.insert(0, '/opt/trn_rl_repo')

import numpy as np
import ml_dtypes

B, C, HH, WW = 16, 128, 32, 32
HW = HH * WW
NH, COND, GROUPS, EPS = 4, 32, 32, 1e-6
SCALE = float(C) ** -0.5
PAIRS = [(0, 1), (1, 0), (2, 3), (3, 2), (0, 2), (2, 0)]  # (q cam, kv cam)
BF = ml_dtypes.bfloat16

_PROG = None


def _build_nc():
    import concourse.bacc as bacc
    import concourse.tile as tile
    import concourse.mybir as mybir
    import concourse.bass_isa as bass_isa

    f32 = mybir.dt.float32
    f32r = mybir.dt.float32r
    bf16 = mybir.dt.bfloat16
    AF = mybir.ActivationFunctionType
    ALU = mybir.AluOpType
    X_AX = mybir.AxisListType.X

    nc = bacc.Bacc("TRN2", target_bir_lowering=False, debug=False, num_devices=8)

    d_x = nc.dram_tensor("x4", [4, C, HW], bf16, kind="ExternalInput")
    d_cqk = nc.dram_tensor("cqk", [16, C, HW], bf16, kind="ExternalInput")
    d_kcp = nc.dram_tensor("kcp", [4, COND + 1, HW], bf16, kind="ExternalInput")
    d_w2c = nc.dram_tensor("w2c", [COND + 1, 256], bf16, kind="ExternalInput")
    d_wqk = nc.dram_tensor("wqk", [C, 512], bf16, kind="ExternalInput")
    d_w2 = nc.dram_tensor("w2", [C, 256], bf16, kind="ExternalInput")
    d_w3 = nc.dram_tensor("w3", [C, 12 * C], bf16, kind="ExternalInput")
    d_gnv = nc.dram_tensor("gnv", [C, 2], f32, kind="ExternalInput")
    d_gind = nc.dram_tensor("gind", [C, GROUPS], bf16, kind="ExternalInput")
    d_gindT = nc.dram_tensor("gindT", [GROUPS, C], bf16, kind="ExternalInput")
    d_out = nc.dram_tensor("out", [6, C, HW], f32, kind="ExternalOutput")

    with tile.TileContext(nc) as tc, nc.allow_low_precision(reason="bf16 pipeline"):
        import contextlib
        ctx = contextlib.ExitStack()
        with ctx:
            cpool = ctx.enter_context(tc.tile_pool(name="consts", bufs=1))
            xpool = ctx.enter_context(tc.tile_pool(name="xp", bufs=1))
            sqpool = ctx.enter_context(tc.tile_pool(name="sqp", bufs=2))
            stpool = ctx.enter_context(tc.tile_pool(name="stp", bufs=2))
            smpool = ctx.enter_context(tc.tile_pool(name="smp", bufs=2))
            sbpool = ctx.enter_context(tc.tile_pool(name="sbp", bufs=2))
            hpool = ctx.enter_context(tc.tile_pool(name="hp", bufs=1))
            condp = ctx.enter_context(tc.tile_pool(name="condp", bufs=8))
            qkpool = ctx.enter_context(tc.tile_pool(name="qkp", bufs=1))
            vtpool = ctx.enter_context(tc.tile_pool(name="vtp", bufs=1))
            epool = ctx.enter_context(tc.tile_pool(name="ep", bufs=20))
            fpool = ctx.enter_context(tc.tile_pool(name="fp", bufs=9))
            dpool = ctx.enter_context(tc.tile_pool(name="dp", bufs=2))
            opool = ctx.enter_context(tc.tile_pool(name="op", bufs=6))
            apool = ctx.enter_context(tc.tile_pool(name="ap", bufs=2))
            P1 = ctx.enter_context(tc.tile_pool(name="ps1", bufs=2, space="PSUM"))
            PB = ctx.enter_context(tc.tile_pool(name="psb", bufs=1, space="PSUM"))
            PA = ctx.enter_context(tc.tile_pool(name="psa", bufs=1, space="PSUM"))

            # ---- constants (x + GN path first; w3 last) ----
            xt = [None] * 4
            for m in range(4):
                xt[m] = xpool.tile([C, HW], bf16, tag=f"xt{m}", name=f"xt{m}")
                nc.sync.dma_start(xt[m][:], d_x[m])
            gnv = cpool.tile([C, 2], f32, tag="gnv")
            nc.sync.dma_start(gnv[:], d_gnv[:])
            gind = cpool.tile([C, GROUPS], bf16, tag="gind")
            nc.sync.dma_start(gind[:], d_gind[:])
            gindT = cpool.tile([GROUPS, C], bf16, tag="gindT")
            nc.sync.dma_start(gindT[:], d_gindT[:])
            wqk = cpool.tile([C, 512], bf16, tag="wqk")
            nc.sync.dma_start(wqk[:], d_wqk[:])
            w2 = cpool.tile([C, 256], bf16, tag="w2")
            nc.sync.dma_start(w2[:], d_w2[:])
            w2c = cpool.tile([COND + 1, 256], bf16, tag="w2c")
            nc.sync.dma_start(w2c[:], d_w2c[:])
            kcp = [None] * 4
            for m in range(4):
                kcp[m] = cpool.tile([COND + 1, HW], bf16, tag=f"kcp{m}", name=f"kcp{m}")
                nc.sync.dma_start(kcp[m][:], d_kcp[m])
            epst = cpool.tile([GROUPS, 1], f32, tag="epst")
            nc.vector.memset(epst[:], EPS)
            # cond tiles: issue all DMAs now, ordered by first use (cam order);
            # the pool depth paces the stream against drain consumption.
            ct_tiles = {}
            CT_ORDER = [(0, 0, 0), (1, 1, 0), (0, 0, 1), (1, 1, 1),
                        (1, 0, 0), (0, 1, 0), (1, 0, 1), (0, 1, 1),
                        (2, 0, 0), (3, 1, 0), (2, 0, 1), (3, 1, 1),
                        (3, 0, 0), (2, 1, 0), (3, 0, 1), (2, 1, 1)]
            for (m, proj, i) in CT_ORDER:
                ct = condp.tile([C, HW], bf16, tag="ct", name=f"ct{m}{proj}{i}")
                nc.sync.dma_start(ct[:], d_cqk[proj * 8 + m * 2 + i])
                ct_tiles[(m, proj, i)] = ct
            w3 = cpool.tile([C, 12 * C], bf16, tag="w3")
            nc.sync.dma_start(w3[:], d_w3[:])

            # GN small PSUM lives in one acc-shaped tile (regions), freed
            # before the attention-phase acc allocations cycle the same buf.
            gn_ps = PA.tile([C, HW], f32, tag="acc", name="gn_ps")

            # GN per cam: sums on the idle Act engine (Identity/Square +
            # accum_out), rstd = sqrt(1/(var+eps)) so only the sqrt+exp act
            # tables are ever loaded. Emission is per-cam so cam0's chain
            # finishes as early as possible.
            h_t = [None] * 4

            def emit_gn(m):
                st = stpool.tile([C, 2], bf16, tag="st", name=f"st{m}", bufs=4)
                scr = sqpool.tile([C, HW], bf16, tag="sq")
                nc.scalar.activation(scr[:], xt[m][:], AF.Identity, accum_out=st[:, 0:1])
                scr2 = sqpool.tile([C, HW], bf16, tag="sq")
                nc.scalar.activation(scr2[:], xt[m][:], AF.Square, accum_out=st[:, 1:2])
                nc.tensor.matmul(gn_ps[0:GROUPS, 2 * m:2 * m + 2], gind[:], st[:],
                                 start=True, stop=True)
                mu = smpool.tile([GROUPS, 1], f32, tag="mu")
                nc.vector.tensor_copy(mu[:], gn_ps[0:GROUPS, 2 * m:2 * m + 1])
                mu2 = smpool.tile([GROUPS, 1], f32, tag="mu2")
                nc.vector.tensor_tensor(out=mu2[:], in0=mu[:], in1=mu[:], op=ALU.mult)
                vpe = smpool.tile([GROUPS, 1], f32, tag="vpe")
                nc.vector.tensor_tensor(out=vpe[:], in0=gn_ps[0:GROUPS, 2 * m + 1:2 * m + 2],
                                        in1=mu2[:], op=ALU.subtract)
                rv = smpool.tile([GROUPS, 1], f32, tag="rv")
                nc.vector.tensor_scalar_add(out=rv[:], in0=vpe[:], scalar1=epst[:])
                nc.vector.reciprocal(out=rv[:], in_=rv[:])
                rstd = smpool.tile([GROUPS, 1], f32, tag="rstd")
                nc.scalar.activation(rstd[:], rv[:], AF.Sqrt)
                bc_in = smpool.tile([GROUPS, 2], bf16, tag="bcin")
                nc.vector.tensor_copy(bc_in[:, 0:1], rstd[:])
                nc.vector.tensor_copy(bc_in[:, 1:2], mu[:])
                nc.tensor.matmul(gn_ps[:, 16 + 2 * m:18 + 2 * m], gindT[:], bc_in[:],
                                 start=True, stop=True)
                se = sbpool.tile([C, 1], f32, tag="se")
                nc.vector.tensor_tensor(out=se[:], in0=gn_ps[:, 16 + 2 * m:17 + 2 * m],
                                        in1=gnv[:, 0:1], op=ALU.mult)
                ms = smpool.tile([C, 1], f32, tag="ms")
                nc.vector.tensor_tensor(out=ms[:], in0=gn_ps[:, 17 + 2 * m:18 + 2 * m],
                                        in1=se[:], op=ALU.mult)
                be = sbpool.tile([C, 1], f32, tag="be")
                nc.vector.tensor_tensor(out=be[:], in0=gnv[:, 1:2], in1=ms[:], op=ALU.subtract)
                ht = hpool.tile([C, HW], bf16, tag=f"ht{m}", name=f"ht{m}")
                nc.vector.tensor_scalar(out=ht[:], in0=xt[m][:], scalar1=se[:], scalar2=be[:],
                                        op0=ALU.mult, op1=ALU.add)
                h_t[m] = ht

            for _m in range(4):
                emit_gn(_m)

            q_sb = [[None] * 2 for _ in range(4)]
            k_sb = [[None] * 2 for _ in range(4)]
            vt_sb = [[None] * 2 for _ in range(4)]
            drain_rr = [0]

            def emit_nin_qk(m, proj, i, pool, tagn, vt_eng):
                ht = h_t[m]
                ps = pool.tile([C, HW], f32, tag=tagn, name="ps_nin")
                wblk = wqk[:, (proj * 2 + i) * 128:(proj * 2 + i + 1) * 128]
                for hf in range(2):
                    fr = slice(hf * 512, (hf + 1) * 512)
                    nc.tensor.matmul(ps[:, fr], wblk, ht[:, fr], start=True, stop=True)
                ct = ct_tiles[(m, proj, i)]
                dt_t = bf16
                t = qkpool.tile([C, HW], dt_t, tag=f"qk{m}_{proj}_{i}",
                                name=f"qk{m}_{proj}_{i}")
                nc.vector.tensor_tensor(out=t[:], in0=ps[:], in1=ct[:], op=ALU.add)
                (q_sb if proj == 0 else k_sb)[m][i] = t

            def emit_nin_vt(m, i, pool, tagn, vt_eng):
                # vT NIN: transposed roles -> output lands key-partitioned.
                # Cond+bias contraction also on the PE (33-partition padded),
                # so the drain is a plain copy.
                ht = h_t[m]
                ps = pool.tile([128, HW], f32, tag=tagn, name="ps_vt")
                for blk in range(8):
                    fr = slice(blk * 128, (blk + 1) * 128)
                    nc.tensor.matmul(ps[:, fr], ht[:, fr], w2[:, i * 128:(i + 1) * 128],
                                     start=True, stop=False)
                    nc.tensor.matmul(ps[:, fr], kcp[m][:, fr], w2c[:, i * 128:(i + 1) * 128],
                                     start=False, stop=True)
                vt = vtpool.tile([128, HW], bf16, tag=f"vt{m}_{i}", name=f"vt{m}_{i}")
                if vt_eng == 'act':
                    nc.scalar.activation(vt[:], ps[:], AF.Identity)
                else:
                    nc.vector.tensor_copy(vt[:], ps[:])
                vt_sb[m][i] = vt

            # cams 0/1 up front (prologue), rotating 3 PSUM bufs; vt drains on
            # the idle Act engine.
            rot = [(P1, "mm"), (P1, "mm"), (PB, "po")]
            nn = 0
            for (mq, mk) in ((0, 1), (1, 0)):
                for i in range(2):
                    for kind, mm_, pj in (('qk', mq, 0), ('qk', mk, 1), ('vt', mk, None)):
                        pool, tagn = rot[nn % 3]
                        nn += 1
                        if kind == 'qk':
                            emit_nin_qk(mm_, pj, i, pool, tagn, None)
                        else:
                            emit_nin_vt(mm_, i, pool, tagn, 'act' if mm_ == 1 else 'dve')

            # cams 2/3: deferred into attention slots 0-3 (PA + po PSUM slots,
            # which are free until the first W3 closes at slot 4). Ordered by
            # first use: pair 2 needs (c2 q, c3 k/vt); pair 3 the reverse.
            deferred = []
            for (mq, mk) in ((2, 3), (3, 2)):
                for i in range(2):
                    deferred.append(lambda pool, tagn, mq=mq, i=i:
                                    emit_nin_qk(mq, 0, i, pool, tagn, None))
                    deferred.append(lambda pool, tagn, mk=mk, i=i:
                                    emit_nin_qk(mk, 1, i, pool, tagn, None))
                    deferred.append(lambda pool, tagn, mk=mk, i=i:
                                    emit_nin_vt(mk, i, pool, tagn, 'pool'))

            # ---- attention: 6 pairs x 2 heads, software-pipelined ----
            # Per slot u: scores/exp/folds of unit u interleaved (on the PE
            # stream) with AV chunks of unit u-1, so the PE fills the gaps
            # while Act paces the pipeline. W3+accumulate+drain close per
            # PAIR on a schedule that keeps the single acc PSUM buf free
            # during slots 0-3 (used by the deferred NIN).
            osb = {}

            def close_pair(pr, last_osb=None):
                accp = PA.tile([C, HW], f32, tag="acc", name=f"acc{pr}")
                for uu in (2 * pr, 2 * pr + 1):
                    ob = osb[uu] if last_osb is None or uu != 2 * pr + 1 else last_osb
                    w3u = w3[:, uu * 128:(uu + 1) * 128]
                    st, sp = uu == 2 * pr, uu == 2 * pr + 1
                    nc.tensor.matmul(accp[:, 0:512], w3u, ob[:, 0:512], start=st, stop=sp,
                                     skip_group_check=True)
                    nc.tensor.matmul(accp[:, 512:1024], w3u, ob[:, 512:1024], start=st, stop=sp,
                                     skip_group_check=True)
                asb = apool.tile([C, HW], f32, tag="asb", name=f"asb{pr}")
                nc.vector.tensor_copy(asb[:], accp[:])
                nc.sync.dma_start(d_out[pr], asb[:])

            def emit_slot(u, prev, hooks, w3_pairs, self_tail=False):
                p, i = u // 2, u % 2
                qc, kc = PAIRS[p]
                qs, ks = q_sb[qc][i], k_sb[kc][i]
                if prev is not None:
                    pu, pE, pdbc = prev
                    pvts = vt_sb[PAIRS[pu // 2][1]][pu % 2]
                    ps_o = PB.tile([C, HW], f32, tag="po", name="ps_o")

                def av_chunk(k0, k1):
                    if prev is None:
                        return
                    for kt in range(k0, k1):
                        st, sp = kt == 0, kt == 7
                        lhs = pvts[:, kt * 128:(kt + 1) * 128]
                        nc.tensor.matmul(ps_o[:, 0:512], lhs, pE[kt][:, 0:512], start=st, stop=sp)
                        nc.tensor.matmul(ps_o[:, 512:1024], lhs, pE[kt][:, 512:1024],
                                         start=st, stop=sp)

                E = []
                fs = {}

                def sc(kt):
                    ps_s = P1.tile([C, HW], f32, tag="mm", name="ps_s")
                    lhs = ks[:, kt * 128:(kt + 1) * 128]
                    nc.tensor.matmul(ps_s[:, 0:512], lhs, qs[:, 0:512], start=True, stop=True)
                    nc.tensor.matmul(ps_s[:, 512:1024], lhs, qs[:, 512:1024], start=True, stop=True)
                    e_t = epool.tile([C, HW], bf16, tag="et")
                    nc.scalar.activation(e_t[:], ps_s[:], AF.Exp, scale=SCALE)
                    E.append(e_t)

                def fold(a, b, dst=None, eng=None):
                    e = eng or nc.vector
                    if dst is None:
                        dst = fpool.tile([C, HW], bf16, tag="f", name="fold")
                        e.tensor_tensor(out=dst[:], in0=a[:], in1=b[:], op=ALU.add)
                    else:
                        e.tensor_tensor(out=dst[:], in0=dst[:], in1=a[:], op=ALU.add)
                    return dst

                sc(0)
                sc(1)
                fs['f01'] = fold(E[0], E[1])
                if len(hooks) > 0:
                    hooks[0](PA, "acc")
                av_chunk(0, 3)
                sc(2)
                sc(3)
                fs['f23'] = fold(E[2], E[3], eng=nc.gpsimd)
                fold(fs['f23'], None, dst=fs['f01'])
                av_chunk(3, 6)
                sc(4)
                sc(5)
                fs['f45'] = fold(E[4], E[5], eng=nc.gpsimd)
                av_chunk(6, 8)
                o_sb = None
                if prev is not None:
                    o_sb = opool.tile([C, HW], bf16, tag="osb", name="o_sb")
                    nc.vector.tensor_tensor(out=o_sb[:], in0=ps_o[:], in1=pdbc[:], op=ALU.mult)
                    osb[pu] = o_sb
                if len(hooks) > 1:
                    hooks[1](PB, "po")
                if self_tail:
                    ps_os = PB.tile([C, HW], f32, tag="po", name="ps_os")

                    def av_self(k0, k1):
                        for kt in range(k0, k1):
                            st, sp = kt == 0, kt == 7
                            lhs = vt_sb[kc][i][:, kt * 128:(kt + 1) * 128]
                            nc.tensor.matmul(ps_os[:, 0:512], lhs, E[kt][:, 0:512],
                                             start=st, stop=sp)
                            nc.tensor.matmul(ps_os[:, 512:1024], lhs, E[kt][:, 512:1024],
                                             start=st, stop=sp)
                    av_self(0, 4)
                sc(6)
                if self_tail:
                    av_self(4, 6)
                sc(7)
                fs['f67'] = fold(E[6], E[7])
                fold(fs['f67'], None, dst=fs['f45'])
                fold(fs['f45'], None, dst=fs['f01'])
                dbc = dpool.tile([C, HW], f32, tag="dbc", name="dbc")
                nc.gpsimd.partition_all_reduce(dbc[:], fs['f01'][:], channels=128,
                                               reduce_op=bass_isa.ReduceOp.add)
                nc.vector.reciprocal(out=dbc[:], in_=dbc[:])
                if len(hooks) > 2:
                    hooks[2](PA, "acc")
                for pr in w3_pairs:
                    close_pair(pr)
                if self_tail:
                    av_self(6, 8)
                    o_sbs = opool.tile([C, HW], bf16, tag="osb", name="o_sbs")
                    nc.vector.tensor_tensor(out=o_sbs[:], in0=ps_os[:], in1=dbc[:], op=ALU.mult)
                    close_pair(5, last_osb=o_sbs)
                return E, dbc

            W3_SCHED = {4: [0], 5: [1], 6: [2], 8: [3], 10: [4]}
            pend = None
            for u in range(12):
                hooks = deferred[2 * u:2 * u + 2] if u < 6 else []
                E, dbc = emit_slot(u, pend, hooks, W3_SCHED.get(u, []),
                                   self_tail=(u == 11))
                pend = (u, E, dbc)

    nc.compile()
    return nc


def _get_prog():
    global _PROG
    if _PROG is None:
        _PROG = _build_nc()
    return _PROG


def _pack_host(x, q_cond, k_a_cond, k_b_cond, gn_scale, gn_bias,
               W0, b0, W1, b1, W2, b2, W3, b3):
    f4 = np.float32
    x = np.ascontiguousarray(x, f4).reshape(B, C, HW)
    q_cs = np.repeat(np.ascontiguousarray(q_cond, f4).reshape(B // 2, COND, HW), 2, axis=0)
    k_cs = np.stack([np.ascontiguousarray(k_a_cond, f4).reshape(B // 2, COND, HW),
                     np.ascontiguousarray(k_b_cond, f4).reshape(B // 2, COND, HW)],
                    axis=1).reshape(B, COND, HW)

    # cond contributions (+bias), f32 compute then bf16
    # q/k: [B, head, C, HW] for the 4 heads; vT: [B, head, HW, C] -> repacked
    condq = np.einsum('bcq,ck->bkq', q_cs, W0[C:].astype(f4)) + b0[None, :, None]
    condk = np.einsum('bcq,ck->bkq', k_cs, W1[C:].astype(f4)) + b1[None, :, None]
    condq = condq.reshape(B, NH, C, HW)
    condk = condk.reshape(B, NH, C, HW)

    gind = np.zeros((C, GROUPS), f4)
    for c in range(C):
        gind[c, c // (C // GROUPS)] = 1.0 / (C // GROUPS * HW)
    gindT = np.zeros((GROUPS, C), f4)
    for c in range(C):
        gindT[c // (C // GROUPS), c] = 1.0
    gnv = np.stack([np.asarray(gn_scale, f4), np.asarray(gn_bias, f4)], axis=1)

    in_maps = []
    for core in range(8):
        g, s = core // 2, core % 2
        hsel = [2 * s, 2 * s + 1]
        cams = [4 * g + m for m in range(4)]
        x4 = x[cams].astype(BF)
        cqk = np.empty((16, C, HW), BF)
        for m in range(4):
            for i in range(2):
                cqk[m * 2 + i] = condq[cams[m], hsel[i]].astype(BF)
                cqk[8 + m * 2 + i] = condk[cams[m], hsel[i]].astype(BF)
        kcp = np.ones((4, COND + 1, HW), f4)
        kcp[:, :COND] = k_cs[cams]
        w2c = np.zeros((COND + 1, 256), f4)
        for i in range(2):
            w2c[:COND, i * 128:(i + 1) * 128] = W2[C:, 128 * hsel[i]:128 * hsel[i] + 128]
            w2c[COND, i * 128:(i + 1) * 128] = b2[128 * hsel[i]:128 * hsel[i] + 128]
        wqk = np.concatenate([W0[:C, 128 * hsel[0]:128 * hsel[0] + 128],
                              W0[:C, 128 * hsel[1]:128 * hsel[1] + 128],
                              W1[:C, 128 * hsel[0]:128 * hsel[0] + 128],
                              W1[:C, 128 * hsel[1]:128 * hsel[1] + 128]], axis=1).astype(BF)
        w2m = np.concatenate([W2[:C, 128 * hsel[0]:128 * hsel[0] + 128],
                              W2[:C, 128 * hsel[1]:128 * hsel[1] + 128]], axis=1).astype(BF)
        w3l = np.zeros((C, 12 * C), f4)
        for p in range(6):
            for i in range(2):
                u = p * 2 + i
                ch = 512 * p + 128 * hsel[i]
                r = ch % 768
                w3l[:, u * C:(u + 1) * C] = W3[r:r + C, :]
        in_maps.append({
            "x4": x4, "cqk": cqk, "kcp": kcp.astype(BF), "w2c": w2c.astype(BF),
            "wqk": wqk, "w2": w2m, "w3": w3l.astype(BF),
            "gnv": gnv, "gind": gind.astype(BF), "gindT": gindT.astype(BF),
        })
    return in_maps


def _assemble(results, x, b3):
    x = np.ascontiguousarray(x, np.float32)
    out = x + np.asarray(b3, np.float32)[None, :, None, None]
    for core in range(8):
        g, s = core // 2, core % 2
        o = results[core]["out"].reshape(6, C, HH, WW)
        for p in range(6):
            j = (512 * p + 256 * s) // 768
            out[4 * g + j] += o[p]
    return out


def kernel(**inputs):
    from concourse.bass_utils import run_bass_kernel_spmd
    nc = _get_prog()
    ins = {k: np.asarray(v) for k, v in inputs.items()}
    in_maps = _pack_host(**ins)
    res = run_bass_kernel_spmd(nc, in_maps, core_ids=list(range(8)))
    return _assemble(res.results, ins["x"], ins["b3"])


# revision 38
# speedup vs baseline: 1.5980x; 1.0314x over previous
"""Trainium2 Bass kernel for nn_CrossAttnBlockppTwoCams.

Sharding: 8 cores = 4 scene-groups x 2 head-halves. Core (g, s) handles scene
group g (batch entries 4g..4g+3) and heads {2s, 2s+1} of all 6 cross-camera
attention pairs -> 12 attention units per core. Each core emits 6 per-pair
partial accumulators (post-W3); the host sums them into the 4 output entries
of the group (each entry receives 1.5 pairs' worth of channels).

Device pipeline per core:
  - GroupNorm for the 4 cameras (bf16 x, stats via DVE reduces + tiny PE
    matmuls, Ln/Exp batched over cams so only 2 act-table loads happen).
  - NIN projections use only the 128-channel h contraction on the PE; the
    32-channel cond contraction plus biases are precomputed on the host and
    added during the PSUM->SBUF drain (DVE/Pool), so no 32-partition matmuls.
  - V is projected directly in transposed (key-partitioned) layout by
    swapping matmul roles, eliminating PE transposes.
  - Attention: f32r q/k scores (8x[128,1024] PSUM tiles), exp on Act (bf16,
    the only large Act work: 96 x 1024-col tiles), softmax denominator via a
    bf16 pairwise tree on DVE + GPSIMD partition_all_reduce (fused
    reduce+broadcast, no PE/PSUM involvement), normalization via DVE divide,
    bf16 AV, W3 accumulated in PSUM across the 2 heads of a pair.
  - Emission is software-pipelined: scores of unit u+1 are emitted before
    AV/W3 of unit u so the PE never waits on the (rate-limiting) Act engine.
"""
import sys

sys.path.insert(0, '/opt/trn_rl_repo')

import numpy as np
import ml_dtypes

B, C, HH, WW = 16, 128, 32, 32
HW = HH * WW
NH, COND, GROUPS, EPS = 4, 32, 32, 1e-6
SCALE = float(C) ** -0.5
PAIRS = [(0, 1), (1, 0), (2, 3), (3, 2), (0, 2), (2, 0)]  # (q cam, kv cam)
BF = ml_dtypes.bfloat16

_PROG = None


def _build_nc():
    import concourse.bacc as bacc
    import concourse.tile as tile
    import concourse.mybir as mybir
    import concourse.bass_isa as bass_isa

    f32 = mybir.dt.float32
    f32r = mybir.dt.float32r
    bf16 = mybir.dt.bfloat16
    AF = mybir.ActivationFunctionType
    ALU = mybir.AluOpType
    X_AX = mybir.AxisListType.X

    nc = bacc.Bacc("TRN2", target_bir_lowering=False, debug=False, num_devices=8)

    d_x = nc.dram_tensor("x4", [4, C, HW], bf16, kind="ExternalInput")
    d_qcp = nc.dram_tensor("qcp", [4, COND + 1, HW], bf16, kind="ExternalInput")
    d_wqkc = nc.dram_tensor("wqkc", [COND + 1, 512], bf16, kind="ExternalInput")
    d_kcp = nc.dram_tensor("kcp", [4, COND + 1, HW], bf16, kind="ExternalInput")
    d_w2c = nc.dram_tensor("w2c", [COND + 1, 256], bf16, kind="ExternalInput")
    d_wqk = nc.dram_tensor("wqk", [C, 512], bf16, kind="ExternalInput")
    d_w2 = nc.dram_tensor("w2", [C, 256], bf16, kind="ExternalInput")
    d_w3 = nc.dram_tensor("w3", [C, 12 * C], bf16, kind="ExternalInput")
    d_gnv = nc.dram_tensor("gnv", [C, 2], f32, kind="ExternalInput")
    d_gind = nc.dram_tensor("gind", [C, GROUPS], bf16, kind="ExternalInput")
    d_gindT = nc.dram_tensor("gindT", [GROUPS, C], bf16, kind="ExternalInput")
    d_out = nc.dram_tensor("out", [6, C, HW], f32, kind="ExternalOutput")

    with tile.TileContext(nc) as tc, nc.allow_low_precision(reason="bf16 pipeline"):
        import contextlib
        ctx = contextlib.ExitStack()
        with ctx:
            cpool = ctx.enter_context(tc.tile_pool(name="consts", bufs=1))
            xpool = ctx.enter_context(tc.tile_pool(name="xp", bufs=1))
            sqpool = ctx.enter_context(tc.tile_pool(name="sqp", bufs=2))
            stpool = ctx.enter_context(tc.tile_pool(name="stp", bufs=2))
            smpool = ctx.enter_context(tc.tile_pool(name="smp", bufs=2))
            sbpool = ctx.enter_context(tc.tile_pool(name="sbp", bufs=2))
            hpool = ctx.enter_context(tc.tile_pool(name="hp", bufs=1))
            qkpool = ctx.enter_context(tc.tile_pool(name="qkp", bufs=1))
            vtpool = ctx.enter_context(tc.tile_pool(name="vtp", bufs=1))
            epool = ctx.enter_context(tc.tile_pool(name="ep", bufs=20))
            fpool = ctx.enter_context(tc.tile_pool(name="fp", bufs=9))
            dpool = ctx.enter_context(tc.tile_pool(name="dp", bufs=2))
            opool = ctx.enter_context(tc.tile_pool(name="op", bufs=6))
            apool = ctx.enter_context(tc.tile_pool(name="ap", bufs=2))
            P1 = ctx.enter_context(tc.tile_pool(name="ps1", bufs=2, space="PSUM"))
            PB = ctx.enter_context(tc.tile_pool(name="psb", bufs=1, space="PSUM"))
            PA = ctx.enter_context(tc.tile_pool(name="psa", bufs=1, space="PSUM"))

            # ---- constants (x + GN path first; w3 last) ----
            xt = [None] * 4
            for m in range(4):
                xt[m] = xpool.tile([C, HW], bf16, tag=f"xt{m}", name=f"xt{m}")
                nc.sync.dma_start(xt[m][:], d_x[m])
            gnv = cpool.tile([C, 2], f32, tag="gnv")
            nc.sync.dma_start(gnv[:], d_gnv[:])
            gind = cpool.tile([C, GROUPS], bf16, tag="gind")
            nc.sync.dma_start(gind[:], d_gind[:])
            gindT = cpool.tile([GROUPS, C], bf16, tag="gindT")
            nc.sync.dma_start(gindT[:], d_gindT[:])
            wqk = cpool.tile([C, 512], bf16, tag="wqk")
            nc.sync.dma_start(wqk[:], d_wqk[:])
            wqkc = cpool.tile([COND + 1, 512], bf16, tag="wqkc")
            nc.sync.dma_start(wqkc[:], d_wqkc[:])
            w2 = cpool.tile([C, 256], bf16, tag="w2")
            nc.sync.dma_start(w2[:], d_w2[:])
            w2c = cpool.tile([COND + 1, 256], bf16, tag="w2c")
            nc.sync.dma_start(w2c[:], d_w2c[:])
            kcp = [None] * 4
            qcp = [None] * 4
            for m in range(4):
                kcp[m] = cpool.tile([COND + 1, HW], bf16, tag=f"kcp{m}", name=f"kcp{m}")
                nc.sync.dma_start(kcp[m][:], d_kcp[m])
                qcp[m] = cpool.tile([COND + 1, HW], bf16, tag=f"qcp{m}", name=f"qcp{m}")
                nc.sync.dma_start(qcp[m][:], d_qcp[m])
            epst = cpool.tile([GROUPS, 1], f32, tag="epst")
            nc.vector.memset(epst[:], EPS)
            w3 = cpool.tile([C, 12 * C], bf16, tag="w3")
            nc.sync.dma_start(w3[:], d_w3[:])

            # GN small PSUM lives in one acc-shaped tile (regions), freed
            # before the attention-phase acc allocations cycle the same buf.
            gn_ps = PA.tile([C, HW], f32, tag="acc", name="gn_ps")

            # GN per cam: sums on the idle Act engine (Identity/Square +
            # accum_out), rstd = sqrt(1/(var+eps)) so only the sqrt+exp act
            # tables are ever loaded. Emission is per-cam so cam0's chain
            # finishes as early as possible.
            h_t = [None] * 4

            def emit_gn(m):
                st = stpool.tile([C, 2], bf16, tag="st", name=f"st{m}", bufs=4)
                nc.vector.tensor_reduce(out=st[:, 0:1], in_=xt[m][:], axis=X_AX, op=ALU.add)
                scr2 = sqpool.tile([C, HW], bf16, tag="sq")
                nc.scalar.activation(scr2[:], xt[m][:], AF.Square, accum_out=st[:, 1:2])
                nc.tensor.matmul(gn_ps[0:GROUPS, 2 * m:2 * m + 2], gind[:], st[:],
                                 start=True, stop=True)
                mu = smpool.tile([GROUPS, 1], f32, tag="mu")
                nc.vector.tensor_copy(mu[:], gn_ps[0:GROUPS, 2 * m:2 * m + 1])
                mu2 = smpool.tile([GROUPS, 1], f32, tag="mu2")
                nc.vector.tensor_tensor(out=mu2[:], in0=mu[:], in1=mu[:], op=ALU.mult)
                vpe = smpool.tile([GROUPS, 1], f32, tag="vpe")
                nc.vector.tensor_tensor(out=vpe[:], in0=gn_ps[0:GROUPS, 2 * m + 1:2 * m + 2],
                                        in1=mu2[:], op=ALU.subtract)
                rv = smpool.tile([GROUPS, 1], f32, tag="rv")
                nc.vector.tensor_scalar_add(out=rv[:], in0=vpe[:], scalar1=epst[:])
                nc.vector.reciprocal(out=rv[:], in_=rv[:])
                rstd = smpool.tile([GROUPS, 1], f32, tag="rstd")
                nc.scalar.activation(rstd[:], rv[:], AF.Sqrt)
                bc_in = smpool.tile([GROUPS, 2], bf16, tag="bcin")
                nc.vector.tensor_copy(bc_in[:, 0:1], rstd[:])
                nc.vector.tensor_copy(bc_in[:, 1:2], mu[:])
                nc.tensor.matmul(gn_ps[:, 16 + 2 * m:18 + 2 * m], gindT[:], bc_in[:],
                                 start=True, stop=True)
                se = sbpool.tile([C, 1], f32, tag="se")
                nc.vector.tensor_tensor(out=se[:], in0=gn_ps[:, 16 + 2 * m:17 + 2 * m],
                                        in1=gnv[:, 0:1], op=ALU.mult)
                ms = smpool.tile([C, 1], f32, tag="ms")
                nc.vector.tensor_tensor(out=ms[:], in0=gn_ps[:, 17 + 2 * m:18 + 2 * m],
                                        in1=se[:], op=ALU.mult)
                be = sbpool.tile([C, 1], f32, tag="be")
                nc.vector.tensor_tensor(out=be[:], in0=gnv[:, 1:2], in1=ms[:], op=ALU.subtract)
                ht = hpool.tile([C, HW], bf16, tag=f"ht{m}", name=f"ht{m}")
                nc.vector.tensor_scalar(out=ht[:], in0=xt[m][:], scalar1=se[:], scalar2=be[:],
                                        op0=ALU.mult, op1=ALU.add)
                h_t[m] = ht

            for _m in range(4):
                emit_gn(_m)

            q_sb = [[None] * 2 for _ in range(4)]
            k_sb = [[None] * 2 for _ in range(4)]
            vt_sb = [[None] * 2 for _ in range(4)]
            drain_rr = [0]

            def emit_nin_qk(m, proj, i, pool, tagn, eng):
                ht = h_t[m]
                cp = qcp[m] if proj == 0 else kcp[m]
                ps = pool.tile([C, HW], f32, tag=tagn, name="ps_nin")
                wblk = wqk[:, (proj * 2 + i) * 128:(proj * 2 + i + 1) * 128]
                wcblk = wqkc[:, (proj * 2 + i) * 128:(proj * 2 + i + 1) * 128]
                for hf in range(2):
                    fr = slice(hf * 512, (hf + 1) * 512)
                    nc.tensor.matmul(ps[:, fr], wblk, ht[:, fr], start=True, stop=False)
                    nc.tensor.matmul(ps[:, fr], wcblk, cp[:, fr], start=False, stop=True)
                t = qkpool.tile([C, HW], bf16, tag=f"qk{m}_{proj}_{i}",
                                name=f"qk{m}_{proj}_{i}")
                if eng == 'act':
                    nc.scalar.activation(t[:], ps[:], AF.Identity)
                else:
                    nc.vector.tensor_copy(t[:], ps[:])
                (q_sb if proj == 0 else k_sb)[m][i] = t

            def emit_nin_vt(m, i, pool, tagn, vt_eng):
                # vT NIN: transposed roles -> output lands key-partitioned.
                # Cond+bias contraction also on the PE (33-partition padded),
                # so the drain is a plain copy.
                ht = h_t[m]
                ps = pool.tile([128, HW], f32, tag=tagn, name="ps_vt")
                for blk in range(8):
                    fr = slice(blk * 128, (blk + 1) * 128)
                    nc.tensor.matmul(ps[:, fr], ht[:, fr], w2[:, i * 128:(i + 1) * 128],
                                     start=True, stop=False)
                    nc.tensor.matmul(ps[:, fr], kcp[m][:, fr], w2c[:, i * 128:(i + 1) * 128],
                                     start=False, stop=True)
                vt = vtpool.tile([128, HW], bf16, tag=f"vt{m}_{i}", name=f"vt{m}_{i}")
                if vt_eng == 'act':
                    nc.scalar.activation(vt[:], ps[:], AF.Identity)
                else:
                    nc.vector.tensor_copy(vt[:], ps[:])

                vt_sb[m][i] = vt

            # cams 0/1 up front (prologue), rotating 3 PSUM bufs; vt drains on
            # the idle Act engine.
            rot = [(P1, "mm"), (P1, "mm"), (PB, "po")]
            nn = 0
            for (mq, mk) in ((0, 1), (1, 0)):
                for i in range(2):
                    for kind, mm_, pj in (('qk', mq, 0), ('qk', mk, 1), ('vt', mk, None)):
                        pool, tagn = rot[nn % 3]
                        nn += 1
                        if kind == 'qk':
                            emit_nin_qk(mm_, pj, i, pool, tagn,
                                        'act' if nn % 2 == 0 else 'dve')
                        else:
                            emit_nin_vt(mm_, i, pool, tagn, 'act' if mm_ == 1 else 'dve')

            # cams 2/3: deferred into attention slots 0-3 (PA + po PSUM slots,
            # which are free until the first W3 closes at slot 4). Ordered by
            # first use: pair 2 needs (c2 q, c3 k/vt); pair 3 the reverse.
            deferred = []
            for (mq, mk) in ((2, 3), (3, 2)):
                for i in range(2):
                    deferred.append(lambda pool, tagn, mq=mq, i=i:
                                    emit_nin_qk(mq, 0, i, pool, tagn, 'dve'))
                    deferred.append(lambda pool, tagn, mk=mk, i=i:
                                    emit_nin_qk(mk, 1, i, pool, tagn, 'dve'))
                    deferred.append(lambda pool, tagn, mk=mk, i=i:
                                    emit_nin_vt(mk, i, pool, tagn, 'act'))

            # ---- attention: 6 pairs x 2 heads, software-pipelined ----
            # Per slot u: scores/exp/folds of unit u interleaved (on the PE
            # stream) with AV chunks of unit u-1, so the PE fills the gaps
            # while Act paces the pipeline. W3+accumulate+drain close per
            # PAIR on a schedule that keeps the single acc PSUM buf free
            # during slots 0-3 (used by the deferred NIN).
            osb = {}

            def close_pair(pr, last_osb=None):
                accp = PA.tile([C, HW], f32, tag="acc", name=f"acc{pr}")
                for uu in (2 * pr, 2 * pr + 1):
                    ob = osb[uu] if last_osb is None or uu != 2 * pr + 1 else last_osb
                    w3u = w3[:, uu * 128:(uu + 1) * 128]
                    st, sp = uu == 2 * pr, uu == 2 * pr + 1
                    nc.tensor.matmul(accp[:, 0:512], w3u, ob[:, 0:512], start=st, stop=sp,
                                     skip_group_check=True)
                    nc.tensor.matmul(accp[:, 512:1024], w3u, ob[:, 512:1024], start=st, stop=sp,
                                     skip_group_check=True)
                asb = apool.tile([C, HW], f32, tag="asb", name=f"asb{pr}")
                nc.vector.tensor_copy(asb[:], accp[:])
                nc.sync.dma_start(d_out[pr], asb[:])

            def emit_slot(u, prev, hooks, w3_pairs, self_tail=False):
                p, i = u // 2, u % 2
                qc, kc = PAIRS[p]
                qs, ks = q_sb[qc][i], k_sb[kc][i]
                if prev is not None:
                    pu, pE, pdbc = prev
                    pvts = vt_sb[PAIRS[pu // 2][1]][pu % 2]
                    ps_o = PB.tile([C, HW], f32, tag="po", name="ps_o")

                def av_chunk(k0, k1):
                    if prev is None:
                        return
                    for kt in range(k0, k1):
                        st, sp = kt == 0, kt == 7
                        lhs = pvts[:, kt * 128:(kt + 1) * 128]
                        nc.tensor.matmul(ps_o[:, 0:512], lhs, pE[kt][:, 0:512], start=st, stop=sp)
                        nc.tensor.matmul(ps_o[:, 512:1024], lhs, pE[kt][:, 512:1024],
                                         start=st, stop=sp)

                E = []
                fs = {}

                def sc(kt):
                    ps_s = P1.tile([C, HW], f32, tag="mm", name="ps_s")
                    lhs = ks[:, kt * 128:(kt + 1) * 128]
                    nc.tensor.matmul(ps_s[:, 0:512], lhs, qs[:, 0:512], start=True, stop=True)
                    nc.tensor.matmul(ps_s[:, 512:1024], lhs, qs[:, 512:1024], start=True, stop=True)
                    e_t = epool.tile([C, HW], bf16, tag="et")
                    nc.scalar.activation(e_t[:], ps_s[:], AF.Exp, scale=SCALE)
                    E.append(e_t)

                def fold(a, b, dst=None, eng=None):
                    e = eng or nc.vector
                    if dst is None:
                        dst = fpool.tile([C, HW], bf16, tag="f", name="fold")
                        e.tensor_tensor(out=dst[:], in0=a[:], in1=b[:], op=ALU.add)
                    else:
                        e.tensor_tensor(out=dst[:], in0=dst[:], in1=a[:], op=ALU.add)
                    return dst

                sc(0)
                sc(1)
                fs['f01'] = fold(E[0], E[1])
                if len(hooks) > 0:
                    hooks[0](PA, "acc")
                av_chunk(0, 3)
                sc(2)
                sc(3)
                fs['f23'] = fold(E[2], E[3], eng=nc.gpsimd)
                fold(fs['f23'], None, dst=fs['f01'])
                av_chunk(3, 6)
                sc(4)
                sc(5)
                fs['f45'] = fold(E[4], E[5], eng=nc.gpsimd)
                av_chunk(6, 8)
                o_sb = None
                if prev is not None:
                    o_sb = opool.tile([C, HW], bf16, tag="osb", name="o_sb")
                    nc.vector.tensor_tensor(out=o_sb[:], in0=ps_o[:], in1=pdbc[:], op=ALU.mult)
                    osb[pu] = o_sb
                if len(hooks) > 1:
                    hooks[1](PB, "po")
                if self_tail:
                    ps_os = PB.tile([C, HW], f32, tag="po", name="ps_os")

                    def av_self(k0, k1):
                        for kt in range(k0, k1):
                            st, sp = kt == 0, kt == 7
                            lhs = vt_sb[kc][i][:, kt * 128:(kt + 1) * 128]
                            nc.tensor.matmul(ps_os[:, 0:512], lhs, E[kt][:, 0:512],
                                             start=st, stop=sp)
                            nc.tensor.matmul(ps_os[:, 512:1024], lhs, E[kt][:, 512:1024],
                                             start=st, stop=sp)
                    av_self(0, 4)
                sc(6)
                if self_tail:
                    av_self(4, 6)
                sc(7)
                fs['f67'] = fold(E[6], E[7])
                fold(fs['f67'], None, dst=fs['f45'])
                fold(fs['f45'], None, dst=fs['f01'])
                dbc = dpool.tile([C, HW], f32, tag="dbc", name="dbc")
                nc.gpsimd.partition_all_reduce(dbc[:], fs['f01'][:], channels=128,
                                               reduce_op=bass_isa.ReduceOp.add)
                nc.vector.reciprocal(out=dbc[:], in_=dbc[:])
                if len(hooks) > 2:
                    hooks[2](PA, "acc")
                for pr in w3_pairs:
                    close_pair(pr)
                if self_tail:
                    av_self(6, 8)
                    o_sbs = opool.tile([C, HW], bf16, tag="osb", name="o_sbs")
                    nc.vector.tensor_tensor(out=o_sbs[:], in0=ps_os[:], in1=dbc[:], op=ALU.mult)
                    close_pair(5, last_osb=o_sbs)
                return E, dbc

            W3_SCHED = {4: [0], 5: [1], 6: [2], 8: [3], 10: [4]}
            pend = None
            for u in range(12):
                hooks = deferred[2 * u:2 * u + 2] if u < 6 else []
                E, dbc = emit_slot(u, pend, hooks, W3_SCHED.get(u, []),
                                   self_tail=(u == 11))
                pend = (u, E, dbc)

    nc.compile()
    return nc


def _get_prog():
    global _PROG
    if _PROG is None:
        _PROG = _build_nc()
    return _PROG


def _pack_host(x, q_cond, k_a_cond, k_b_cond, gn_scale, gn_bias,
               W0, b0, W1, b1, W2, b2, W3, b3):
    f4 = np.float32
    x = np.ascontiguousarray(x, f4).reshape(B, C, HW)
    q_cs = np.repeat(np.ascontiguousarray(q_cond, f4).reshape(B // 2, COND, HW), 2, axis=0)
    k_cs = np.stack([np.ascontiguousarray(k_a_cond, f4).reshape(B // 2, COND, HW),
                     np.ascontiguousarray(k_b_cond, f4).reshape(B // 2, COND, HW)],
                    axis=1).reshape(B, COND, HW)


    gind = np.zeros((C, GROUPS), f4)
    for c in range(C):
        gind[c, c // (C // GROUPS)] = 1.0 / (C // GROUPS * HW)
    gindT = np.zeros((GROUPS, C), f4)
    for c in range(C):
        gindT[c // (C // GROUPS), c] = 1.0
    gnv = np.stack([np.asarray(gn_scale, f4), np.asarray(gn_bias, f4)], axis=1)

    in_maps = []
    for core in range(8):
        g, s = core // 2, core % 2
        hsel = [2 * s, 2 * s + 1]
        cams = [4 * g + m for m in range(4)]
        x4 = x[cams].astype(BF)
        kcp = np.ones((4, COND + 1, HW), f4)
        kcp[:, :COND] = k_cs[cams]
        qcp = np.ones((4, COND + 1, HW), f4)
        qcp[:, :COND] = q_cs[cams]
        wqkc = np.zeros((COND + 1, 512), f4)
        for i in range(2):
            cl = slice(128 * hsel[i], 128 * hsel[i] + 128)
            wqkc[:COND, i * 128:(i + 1) * 128] = W0[C:, cl]
            wqkc[COND, i * 128:(i + 1) * 128] = b0[cl]
            wqkc[:COND, 256 + i * 128:256 + (i + 1) * 128] = W1[C:, cl]
            wqkc[COND, 256 + i * 128:256 + (i + 1) * 128] = b1[cl]
        w2c = np.zeros((COND + 1, 256), f4)
        for i in range(2):
            w2c[:COND, i * 128:(i + 1) * 128] = W2[C:, 128 * hsel[i]:128 * hsel[i] + 128]
            w2c[COND, i * 128:(i + 1) * 128] = b2[128 * hsel[i]:128 * hsel[i] + 128]
        wqk = np.concatenate([W0[:C, 128 * hsel[0]:128 * hsel[0] + 128],
                              W0[:C, 128 * hsel[1]:128 * hsel[1] + 128],
                              W1[:C, 128 * hsel[0]:128 * hsel[0] + 128],
                              W1[:C, 128 * hsel[1]:128 * hsel[1] + 128]], axis=1).astype(BF)
        w2m = np.concatenate([W2[:C, 128 * hsel[0]:128 * hsel[0] + 128],
                              W2[:C, 128 * hsel[1]:128 * hsel[1] + 128]], axis=1).astype(BF)
        w3l = np.zeros((C, 12 * C), f4)
        for p in range(6):
            for i in range(2):
                u = p * 2 + i
                ch = 512 * p + 128 * hsel[i]
                r = ch % 768
                w3l[:, u * C:(u + 1) * C] = W3[r:r + C, :]
        in_maps.append({
            "x4": x4, "qcp": qcp.astype(BF), "wqkc": wqkc.astype(BF),
            "kcp": kcp.astype(BF), "w2c": w2c.astype(BF),
            "wqk": wqk, "w2": w2m, "w3": w3l.astype(BF),
            "gnv": gnv, "gind": gind.astype(BF), "gindT": gindT.astype(BF),
        })
    return in_maps


def _assemble(results, x, b3):
    x = np.ascontiguousarray(x, np.float32)
    out = x + np.asarray(b3, np.float32)[None, :, None, None]
    for core in range(8):
        g, s = core // 2, core % 2
        o = results[core]["out"].reshape(6, C, HH, WW)
        for p in range(6):
            j = (512 * p + 256 * s) // 768
            out[4 * g + j] += o[p]
    return out


def kernel(**inputs):
    from concourse.bass_utils import run_bass_kernel_spmd
    nc = _get_prog()
    ins = {k: np.asarray(v) for k, v in inputs.items()}
    in_maps = _pack_host(**ins)
    res = run_bass_kernel_spmd(nc, in_maps, core_ids=list(range(8)))
    return _assemble(res.results, ins["x"], ins["b3"])


# revision 47
# speedup vs baseline: 1.6281x; 1.0189x over previous
"""Trainium2 Bass kernel for nn_CrossAttnBlockppTwoCams.

Sharding: 8 cores = 4 scene-groups x 2 head-halves. Core (g, s) handles scene
group g (batch entries 4g..4g+3) and heads {2s, 2s+1} of all 6 cross-camera
attention pairs -> 12 attention units per core. Each core emits 6 per-pair
partial accumulators (post-W3); the host sums them into the 4 output entries
of the group (each entry receives 1.5 pairs' worth of channels).

Device pipeline per core:
  - GroupNorm for the 4 cameras (bf16 x, stats via DVE reduces + tiny PE
    matmuls, Ln/Exp batched over cams so only 2 act-table loads happen).
  - NIN projections use only the 128-channel h contraction on the PE; the
    32-channel cond contraction plus biases are precomputed on the host and
    added during the PSUM->SBUF drain (DVE/Pool), so no 32-partition matmuls.
  - V is projected directly in transposed (key-partitioned) layout by
    swapping matmul roles, eliminating PE transposes.
  - Attention: f32r q/k scores (8x[128,1024] PSUM tiles), exp on Act (bf16,
    the only large Act work: 96 x 1024-col tiles), softmax denominator via a
    bf16 pairwise tree on DVE + GPSIMD partition_all_reduce (fused
    reduce+broadcast, no PE/PSUM involvement), normalization via DVE divide,
    bf16 AV, W3 accumulated in PSUM across the 2 heads of a pair.
  - Emission is software-pipelined: scores of unit u+1 are emitted before
    AV/W3 of unit u so the PE never waits on the (rate-limiting) Act engine.
"""
import sys

sys.path.insert(0, '/opt/trn_rl_repo')

import numpy as np
import ml_dtypes

B, C, HH, WW = 16, 128, 32, 32
HW = HH * WW
NH, COND, GROUPS, EPS = 4, 32, 32, 1e-6
SCALE = float(C) ** -0.5
PAIRS = [(0, 1), (1, 0), (2, 3), (3, 2), (0, 2), (2, 0)]  # (q cam, kv cam)
BF = ml_dtypes.bfloat16

_PROG = None


def _build_nc():
    import concourse.bacc as bacc
    import concourse.tile as tile
    import concourse.mybir as mybir
    import concourse.bass_isa as bass_isa

    f32 = mybir.dt.float32
    f32r = mybir.dt.float32r
    bf16 = mybir.dt.bfloat16
    AF = mybir.ActivationFunctionType
    ALU = mybir.AluOpType
    X_AX = mybir.AxisListType.X

    nc = bacc.Bacc("TRN2", target_bir_lowering=False, debug=False, num_devices=8)

    d_x = nc.dram_tensor("x4", [4, C, HW], bf16, kind="ExternalInput")
    d_qcp = nc.dram_tensor("qcp", [4, COND + 1, HW], bf16, kind="ExternalInput")
    d_wqkc = nc.dram_tensor("wqkc", [COND + 1, 512], bf16, kind="ExternalInput")
    d_kcp = nc.dram_tensor("kcp", [4, COND + 1, HW], bf16, kind="ExternalInput")
    d_w2c = nc.dram_tensor("w2c", [COND + 1, 256], bf16, kind="ExternalInput")
    d_wqk = nc.dram_tensor("wqk", [C, 512], bf16, kind="ExternalInput")
    d_w2 = nc.dram_tensor("w2", [C, 256], bf16, kind="ExternalInput")
    d_w3 = nc.dram_tensor("w3", [C, 12 * C], bf16, kind="ExternalInput")
    d_gnv = nc.dram_tensor("gnv", [C, 2], f32, kind="ExternalInput")
    d_gind = nc.dram_tensor("gind", [C, GROUPS], bf16, kind="ExternalInput")
    d_gindT = nc.dram_tensor("gindT", [GROUPS, C], bf16, kind="ExternalInput")
    d_out = nc.dram_tensor("out", [6, C, HW], f32, kind="ExternalOutput")

    with tile.TileContext(nc) as tc, nc.allow_low_precision(reason="bf16 pipeline"):
        import contextlib
        ctx = contextlib.ExitStack()
        with ctx:
            cpool = ctx.enter_context(tc.tile_pool(name="consts", bufs=1))
            xpool = ctx.enter_context(tc.tile_pool(name="xp", bufs=1))
            sqpool = ctx.enter_context(tc.tile_pool(name="sqp", bufs=2))
            stpool = ctx.enter_context(tc.tile_pool(name="stp", bufs=2))
            smpool = ctx.enter_context(tc.tile_pool(name="smp", bufs=2))
            sbpool = ctx.enter_context(tc.tile_pool(name="sbp", bufs=2))
            hpool = ctx.enter_context(tc.tile_pool(name="hp", bufs=1))
            qkpool = ctx.enter_context(tc.tile_pool(name="qkp", bufs=1))
            vtpool = ctx.enter_context(tc.tile_pool(name="vtp", bufs=1))
            epool = ctx.enter_context(tc.tile_pool(name="ep", bufs=20))
            fpool = ctx.enter_context(tc.tile_pool(name="fp", bufs=9))
            dpool = ctx.enter_context(tc.tile_pool(name="dp", bufs=2))
            opool = ctx.enter_context(tc.tile_pool(name="op", bufs=6))
            apool = ctx.enter_context(tc.tile_pool(name="ap", bufs=2))
            P1 = ctx.enter_context(tc.tile_pool(name="ps1", bufs=2, space="PSUM"))
            PB = ctx.enter_context(tc.tile_pool(name="psb", bufs=1, space="PSUM"))
            PA = ctx.enter_context(tc.tile_pool(name="psa", bufs=1, space="PSUM"))

            # ---- constants (x + GN path first; w3 last) ----
            xt = [None] * 4
            for m in range(4):
                xt[m] = xpool.tile([C, HW], bf16, tag=f"xt{m}", name=f"xt{m}")
                nc.sync.dma_start(xt[m][:], d_x[m])
            gnv = cpool.tile([C, 2], f32, tag="gnv")
            nc.sync.dma_start(gnv[:], d_gnv[:])
            gind = cpool.tile([C, GROUPS], bf16, tag="gind")
            nc.sync.dma_start(gind[:], d_gind[:])
            gindT = cpool.tile([GROUPS, C], bf16, tag="gindT")
            nc.sync.dma_start(gindT[:], d_gindT[:])
            wqk = cpool.tile([C, 512], bf16, tag="wqk")
            nc.sync.dma_start(wqk[:], d_wqk[:])
            wqkc = cpool.tile([COND + 1, 512], bf16, tag="wqkc")
            nc.sync.dma_start(wqkc[:], d_wqkc[:])
            w2 = cpool.tile([C, 256], bf16, tag="w2")
            nc.sync.dma_start(w2[:], d_w2[:])
            w2c = cpool.tile([COND + 1, 256], bf16, tag="w2c")
            nc.sync.dma_start(w2c[:], d_w2c[:])
            kcp = [None] * 4
            qcp = [None] * 4
            for m in range(4):
                kcp[m] = cpool.tile([COND + 1, HW], bf16, tag=f"kcp{m}", name=f"kcp{m}")
                nc.sync.dma_start(kcp[m][:], d_kcp[m])
                qcp[m] = cpool.tile([COND + 1, HW], bf16, tag=f"qcp{m}", name=f"qcp{m}")
                nc.sync.dma_start(qcp[m][:], d_qcp[m])
            epst = cpool.tile([GROUPS, 1], f32, tag="epst")
            nc.vector.memset(epst[:], EPS)
            w3 = cpool.tile([C, 12 * C], bf16, tag="w3")
            nc.sync.dma_start(w3[:], d_w3[:])

            # GN small PSUM lives in one acc-shaped tile (regions), freed
            # before the attention-phase acc allocations cycle the same buf.
            gn_ps = PA.tile([C, HW], f32, tag="acc", name="gn_ps")

            # GN per cam: sums on the idle Act engine (Identity/Square +
            # accum_out), rstd = sqrt(1/(var+eps)) so only the sqrt+exp act
            # tables are ever loaded. Emission is per-cam so cam0's chain
            # finishes as early as possible.
            h_t = [None] * 4

            def emit_gn(m):
                st = stpool.tile([C, 2], bf16, tag="st", name=f"st{m}", bufs=4)
                nc.vector.tensor_reduce(out=st[:, 0:1], in_=xt[m][:], axis=X_AX, op=ALU.add)
                scr2 = sqpool.tile([C, HW], bf16, tag="sq")
                nc.scalar.activation(scr2[:], xt[m][:], AF.Square, accum_out=st[:, 1:2])
                nc.tensor.matmul(gn_ps[0:GROUPS, 2 * m:2 * m + 2], gind[:], st[:],
                                 start=True, stop=True)
                mu = smpool.tile([GROUPS, 1], f32, tag="mu")
                nc.vector.tensor_copy(mu[:], gn_ps[0:GROUPS, 2 * m:2 * m + 1])
                mu2 = smpool.tile([GROUPS, 1], f32, tag="mu2")
                nc.vector.tensor_tensor(out=mu2[:], in0=mu[:], in1=mu[:], op=ALU.mult)
                vpe = smpool.tile([GROUPS, 1], f32, tag="vpe")
                nc.vector.tensor_tensor(out=vpe[:], in0=gn_ps[0:GROUPS, 2 * m + 1:2 * m + 2],
                                        in1=mu2[:], op=ALU.subtract)
                rv = smpool.tile([GROUPS, 1], f32, tag="rv")
                nc.vector.tensor_scalar_add(out=rv[:], in0=vpe[:], scalar1=epst[:])
                nc.vector.reciprocal(out=rv[:], in_=rv[:])
                rstd = smpool.tile([GROUPS, 1], f32, tag="rstd")
                nc.scalar.activation(rstd[:], rv[:], AF.Sqrt)
                bc_in = smpool.tile([GROUPS, 2], bf16, tag="bcin")
                nc.vector.tensor_copy(bc_in[:, 0:1], rstd[:])
                nc.vector.tensor_copy(bc_in[:, 1:2], mu[:])
                nc.tensor.matmul(gn_ps[:, 16 + 2 * m:18 + 2 * m], gindT[:], bc_in[:],
                                 start=True, stop=True)
                se = sbpool.tile([C, 1], f32, tag="se")
                nc.vector.tensor_tensor(out=se[:], in0=gn_ps[:, 16 + 2 * m:17 + 2 * m],
                                        in1=gnv[:, 0:1], op=ALU.mult)
                ms = smpool.tile([C, 1], f32, tag="ms")
                nc.vector.tensor_tensor(out=ms[:], in0=gn_ps[:, 17 + 2 * m:18 + 2 * m],
                                        in1=se[:], op=ALU.mult)
                be = sbpool.tile([C, 1], f32, tag="be")
                nc.vector.tensor_tensor(out=be[:], in0=gnv[:, 1:2], in1=ms[:], op=ALU.subtract)
                ht = hpool.tile([C, HW], bf16, tag=f"ht{m}", name=f"ht{m}")
                nc.vector.tensor_scalar(out=ht[:], in0=xt[m][:], scalar1=se[:], scalar2=be[:],
                                        op0=ALU.mult, op1=ALU.add)
                h_t[m] = ht

            for _m in range(4):
                emit_gn(_m)

            q_sb = [[None] * 2 for _ in range(4)]
            k_sb = [[None] * 2 for _ in range(4)]
            vt_sb = [[None] * 2 for _ in range(4)]
            drain_rr = [0]

            def emit_nin_qk(m, proj, i, pool, tagn, eng):
                ht = h_t[m]
                cp = qcp[m] if proj == 0 else kcp[m]
                ps = pool.tile([C, HW], f32, tag=tagn, name="ps_nin")
                wblk = wqk[:, (proj * 2 + i) * 128:(proj * 2 + i + 1) * 128]
                wcblk = wqkc[:, (proj * 2 + i) * 128:(proj * 2 + i + 1) * 128]
                for hf in range(2):
                    fr = slice(hf * 512, (hf + 1) * 512)
                    nc.tensor.matmul(ps[:, fr], wblk, ht[:, fr], start=True, stop=False)
                    nc.tensor.matmul(ps[:, fr], wcblk, cp[:, fr], start=False, stop=True)
                t = qkpool.tile([C, HW], bf16, tag=f"qk{m}_{proj}_{i}",
                                name=f"qk{m}_{proj}_{i}")
                if eng == 'act':
                    nc.scalar.activation(t[:], ps[:], AF.Identity)
                else:
                    nc.vector.tensor_copy(t[:], ps[:])
                (q_sb if proj == 0 else k_sb)[m][i] = t

            def emit_nin_vt(m, i, pool, tagn, vt_eng):
                # vT NIN: transposed roles -> output lands key-partitioned.
                # Cond+bias contraction also on the PE (33-partition padded),
                # so the drain is a plain copy.
                ht = h_t[m]
                ps = pool.tile([128, HW], f32, tag=tagn, name="ps_vt")
                for blk in range(8):
                    fr = slice(blk * 128, (blk + 1) * 128)
                    nc.tensor.matmul(ps[:, fr], ht[:, fr], w2[:, i * 128:(i + 1) * 128],
                                     start=True, stop=False)
                    nc.tensor.matmul(ps[:, fr], kcp[m][:, fr], w2c[:, i * 128:(i + 1) * 128],
                                     start=False, stop=True)
                vt = vtpool.tile([128, HW], bf16, tag=f"vt{m}_{i}", name=f"vt{m}_{i}")
                if vt_eng == 'act':
                    nc.scalar.activation(vt[:], ps[:], AF.Identity)
                else:
                    nc.vector.tensor_copy(vt[:], ps[:])

                vt_sb[m][i] = vt

            # cams 0/1 up front (prologue), rotating 3 PSUM bufs; vt drains on
            # the idle Act engine.
            rot = [(P1, "mm"), (P1, "mm"), (PB, "po")]
            nn = 0
            for (mq, mk) in ((0, 1), (1, 0)):
                for i in range(2):
                    for kind, mm_, pj in (('qk', mq, 0), ('qk', mk, 1), ('vt', mk, None)):
                        pool, tagn = rot[nn % 3]
                        nn += 1
                        if kind == 'qk':
                            emit_nin_qk(mm_, pj, i, pool, tagn,
                                        'act' if nn % 2 == 0 else 'dve')
                        else:
                            emit_nin_vt(mm_, i, pool, tagn, 'act' if mm_ == 1 else 'dve')

            # cams 2/3: deferred into attention slots 0-3 (PA + po PSUM slots,
            # which are free until the first W3 closes at slot 4). Ordered by
            # first use: pair 2 needs (c2 q, c3 k/vt); pair 3 the reverse.
            deferred = []
            for (mq, mk) in ((2, 3), (3, 2)):
                for i in range(2):
                    deferred.append(lambda pool, tagn, mq=mq, i=i:
                                    emit_nin_qk(mq, 0, i, pool, tagn, 'act'))
                    deferred.append(lambda pool, tagn, mk=mk, i=i:
                                    emit_nin_qk(mk, 1, i, pool, tagn, 'dve'))
                    deferred.append(lambda pool, tagn, mk=mk, i=i:
                                    emit_nin_vt(mk, i, pool, tagn, 'act'))

            # ---- attention: 6 pairs x 2 heads, software-pipelined ----
            # Per slot u: scores/exp/folds of unit u interleaved (on the PE
            # stream) with AV chunks of unit u-1, so the PE fills the gaps
            # while Act paces the pipeline. W3+accumulate+drain close per
            # PAIR on a schedule that keeps the single acc PSUM buf free
            # during slots 0-3 (used by the deferred NIN).
            osb = {}

            def close_pair(pr, last_osb=None, drain_eng='dve'):
                accp = PA.tile([C, HW], f32, tag="acc", name=f"acc{pr}")
                for uu in (2 * pr, 2 * pr + 1):
                    ob = osb[uu] if last_osb is None or uu != 2 * pr + 1 else last_osb
                    w3u = w3[:, uu * 128:(uu + 1) * 128]
                    st, sp = uu == 2 * pr, uu == 2 * pr + 1
                    nc.tensor.matmul(accp[:, 0:512], w3u, ob[:, 0:512], start=st, stop=sp,
                                     skip_group_check=True)
                    nc.tensor.matmul(accp[:, 512:1024], w3u, ob[:, 512:1024], start=st, stop=sp,
                                     skip_group_check=True)
                asb = apool.tile([C, HW], f32, tag="asb", name=f"asb{pr}")
                if drain_eng == 'act':
                    nc.scalar.activation(asb[:], accp[:], AF.Identity)
                else:
                    nc.vector.tensor_copy(asb[:], accp[:])
                nc.sync.dma_start(d_out[pr], asb[:])

            def emit_slot(u, prev, hooks, w3_pairs, self_tail=False):
                p, i = u // 2, u % 2
                qc, kc = PAIRS[p]
                qs, ks = q_sb[qc][i], k_sb[kc][i]
                if prev is not None:
                    pu, pE, pdbc = prev
                    pvts = vt_sb[PAIRS[pu // 2][1]][pu % 2]
                    ps_o = PB.tile([C, HW], f32, tag="po", name="ps_o")

                def av_chunk(k0, k1):
                    if prev is None:
                        return
                    for kt in range(k0, k1):
                        st, sp = kt == 0, kt == 7
                        lhs = pvts[:, kt * 128:(kt + 1) * 128]
                        nc.tensor.matmul(ps_o[:, 0:512], lhs, pE[kt][:, 0:512], start=st, stop=sp)
                        nc.tensor.matmul(ps_o[:, 512:1024], lhs, pE[kt][:, 512:1024],
                                         start=st, stop=sp)

                E = []
                fs = {}

                def sc(kt):
                    ps_s = P1.tile([C, HW], f32, tag="mm", name="ps_s")
                    lhs = ks[:, kt * 128:(kt + 1) * 128]
                    nc.tensor.matmul(ps_s[:, 0:512], lhs, qs[:, 0:512], start=True, stop=True)
                    nc.tensor.matmul(ps_s[:, 512:1024], lhs, qs[:, 512:1024], start=True, stop=True)
                    e_t = epool.tile([C, HW], bf16, tag="et")
                    nc.scalar.activation(e_t[:], ps_s[:], AF.Exp, scale=SCALE)
                    E.append(e_t)

                def fold(a, b, dst=None, eng=None):
                    e = eng or nc.vector
                    if dst is None:
                        dst = fpool.tile([C, HW], bf16, tag="f", name="fold")
                        e.tensor_tensor(out=dst[:], in0=a[:], in1=b[:], op=ALU.add)
                    else:
                        e.tensor_tensor(out=dst[:], in0=dst[:], in1=a[:], op=ALU.add)
                    return dst

                sc(0)
                sc(1)
                fs['f01'] = fold(E[0], E[1])
                if len(hooks) > 0:
                    hooks[0](PA, "acc")
                av_chunk(0, 3)
                sc(2)
                if self_tail:
                    fold(E[2], None, dst=fs['f01'])
                sc(3)
                if self_tail:
                    fold(E[3], None, dst=fs['f01'])
                else:
                    fs['f23'] = fold(E[2], E[3], eng=nc.gpsimd)
                    fold(fs['f23'], None, dst=fs['f01'])
                av_chunk(3, 6)
                sc(4)
                if self_tail:
                    fold(E[4], None, dst=fs['f01'])
                sc(5)
                if self_tail:
                    fold(E[5], None, dst=fs['f01'])
                else:
                    fs['f45'] = fold(E[4], E[5], eng=nc.gpsimd)
                av_chunk(6, 8)
                o_sb = None
                if prev is not None:
                    o_sb = opool.tile([C, HW], bf16, tag="osb", name="o_sb")
                    nc.vector.tensor_tensor(out=o_sb[:], in0=ps_o[:], in1=pdbc[:], op=ALU.mult)
                    osb[pu] = o_sb
                if len(hooks) > 1:
                    hooks[1](PB, "po")
                if self_tail:
                    ps_os = PB.tile([C, HW], f32, tag="po", name="ps_os")

                    def av_self(k0, k1):
                        for kt in range(k0, k1):
                            st, sp = kt == 0, kt == 7
                            lhs = vt_sb[kc][i][:, kt * 128:(kt + 1) * 128]
                            nc.tensor.matmul(ps_os[:, 0:512], lhs, E[kt][:, 0:512],
                                             start=st, stop=sp)
                            nc.tensor.matmul(ps_os[:, 512:1024], lhs, E[kt][:, 512:1024],
                                             start=st, stop=sp)
                    av_self(0, 4)
                sc(6)
                if self_tail:
                    fold(E[6], None, dst=fs['f01'])
                    av_self(4, 6)
                sc(7)
                if self_tail:
                    fold(E[7], None, dst=fs['f01'])
                else:
                    fs['f67'] = fold(E[6], E[7])
                    fold(fs['f67'], None, dst=fs['f45'])
                    fold(fs['f45'], None, dst=fs['f01'])
                dbc = dpool.tile([C, HW], f32, tag="dbc", name="dbc")
                nc.gpsimd.partition_all_reduce(dbc[:], fs['f01'][:], channels=128,
                                               reduce_op=bass_isa.ReduceOp.add)
                nc.vector.reciprocal(out=dbc[:], in_=dbc[:])
                if len(hooks) > 2:
                    hooks[2](PA, "acc")
                for pr in w3_pairs:
                    close_pair(pr)
                if self_tail:
                    av_self(6, 8)
                    o_sbs = opool.tile([C, HW], bf16, tag="osb", name="o_sbs")
                    nc.vector.tensor_tensor(out=o_sbs[:], in0=ps_os[:], in1=dbc[:], op=ALU.mult)
                    close_pair(5, last_osb=o_sbs, drain_eng='act')
                return E, dbc

            W3_SCHED = {4: [0], 5: [1], 6: [2], 8: [3], 10: [4]}
            pend = None
            for u in range(12):
                hooks = deferred[2 * u:2 * u + 2] if u < 6 else []
                E, dbc = emit_slot(u, pend, hooks, W3_SCHED.get(u, []),
                                   self_tail=(u == 11))
                pend = (u, E, dbc)

    nc.compile()
    return nc


def _get_prog():
    global _PROG
    if _PROG is None:
        _PROG = _build_nc()
    return _PROG


def _pack_host(x, q_cond, k_a_cond, k_b_cond, gn_scale, gn_bias,
               W0, b0, W1, b1, W2, b2, W3, b3):
    f4 = np.float32
    x = np.ascontiguousarray(x, f4).reshape(B, C, HW)
    q_cs = np.repeat(np.ascontiguousarray(q_cond, f4).reshape(B // 2, COND, HW), 2, axis=0)
    k_cs = np.stack([np.ascontiguousarray(k_a_cond, f4).reshape(B // 2, COND, HW),
                     np.ascontiguousarray(k_b_cond, f4).reshape(B // 2, COND, HW)],
                    axis=1).reshape(B, COND, HW)


    gind = np.zeros((C, GROUPS), f4)
    for c in range(C):
        gind[c, c // (C // GROUPS)] = 1.0 / (C // GROUPS * HW)
    gindT = np.zeros((GROUPS, C), f4)
    for c in range(C):
        gindT[c // (C // GROUPS), c] = 1.0
    gnv = np.stack([np.asarray(gn_scale, f4), np.asarray(gn_bias, f4)], axis=1)

    in_maps = []
    for core in range(8):
        g, s = core // 2, core % 2
        hsel = [2 * s, 2 * s + 1]
        cams = [4 * g + m for m in range(4)]
        x4 = x[cams].astype(BF)
        kcp = np.ones((4, COND + 1, HW), f4)
        kcp[:, :COND] = k_cs[cams]
        qcp = np.ones((4, COND + 1, HW), f4)
        qcp[:, :COND] = q_cs[cams]
        wqkc = np.zeros((COND + 1, 512), f4)
        for i in range(2):
            cl = slice(128 * hsel[i], 128 * hsel[i] + 128)
            wqkc[:COND, i * 128:(i + 1) * 128] = W0[C:, cl]
            wqkc[COND, i * 128:(i + 1) * 128] = b0[cl]
            wqkc[:COND, 256 + i * 128:256 + (i + 1) * 128] = W1[C:, cl]
            wqkc[COND, 256 + i * 128:256 + (i + 1) * 128] = b1[cl]
        w2c = np.zeros((COND + 1, 256), f4)
        for i in range(2):
            w2c[:COND, i * 128:(i + 1) * 128] = W2[C:, 128 * hsel[i]:128 * hsel[i] + 128]
            w2c[COND, i * 128:(i + 1) * 128] = b2[128 * hsel[i]:128 * hsel[i] + 128]
        wqk = np.concatenate([W0[:C, 128 * hsel[0]:128 * hsel[0] + 128],
                              W0[:C, 128 * hsel[1]:128 * hsel[1] + 128],
                              W1[:C, 128 * hsel[0]:128 * hsel[0] + 128],
                              W1[:C, 128 * hsel[1]:128 * hsel[1] + 128]], axis=1).astype(BF)
        w2m = np.concatenate([W2[:C, 128 * hsel[0]:128 * hsel[0] + 128],
                              W2[:C, 128 * hsel[1]:128 * hsel[1] + 128]], axis=1).astype(BF)
        w3l = np.zeros((C, 12 * C), f4)
        for p in range(6):
            for i in range(2):
                u = p * 2 + i
                ch = 512 * p + 128 * hsel[i]
                r = ch % 768
                w3l[:, u * C:(u + 1) * C] = W3[r:r + C, :]
        in_maps.append({
            "x4": x4, "qcp": qcp.astype(BF), "wqkc": wqkc.astype(BF),
            "kcp": kcp.astype(BF), "w2c": w2c.astype(BF),
            "wqk": wqk, "w2": w2m, "w3": w3l.astype(BF),
            "gnv": gnv, "gind": gind.astype(BF), "gindT": gindT.astype(BF),
        })
    return in_maps


def _assemble(results, x, b3):
    x = np.ascontiguousarray(x, np.float32)
    out = x + np.asarray(b3, np.float32)[None, :, None, None]
    for core in range(8):
        g, s = core // 2, core % 2
        o = results[core]["out"].reshape(6, C, HH, WW)
        for p in range(6):
            j = (512 * p + 256 * s) // 768
            out[4 * g + j] += o[p]
    return out


def kernel(**inputs):
    from concourse.bass_utils import run_bass_kernel_spmd
    nc = _get_prog()
    ins = {k: np.asarray(v) for k, v in inputs.items()}
    in_maps = _pack_host(**ins)
    res = run_bass_kernel_spmd(nc, in_maps, core_ids=list(range(8)))
    return _assemble(res.results, ins["x"], ins["b3"])
